# revision 1
# baseline (speedup 1.0000x reference)
"""Trainium2 Bass kernel for nn_CaptchaRecognizer (norse-style SNN).

Strategy (pure data-parallel over batch, 8 NeuronCores, 16 images each):

The reference steps t=0..31 through all 6 (LIF -> LILinear) blocks. We reorder
loops to process LAYER BY LAYER so each weight matrix streams from HBM once:

  stage 0:  the encoder resets to exactly 0 on spike, so its spike train is
            periodic and the encoder+LIF0 cascade is a piecewise-constant
            function of x alone. Host precomputes the fp32-exact breakpoints
            and per-class spike patterns (4 breakpoints, 13 active timesteps);
            the device builds an integer spike-code per element with 4
            compare-accumulate ops and bit-extracts the spike planes —
            bit-exact vs the reference recurrence, interleaved in chunk groups
            with the layer-0 matmul.
  matmul k: J_k = S_k @ w_k^T with K-accumulation in PSUM (bf16 operands,
            fp32 accumulate). Weights are host-transposed/padded/cast; layer-0
            drains permute PSUM (t,b) columns into (b,t) SBUF tiles.
  LI cell:  two segmented tensor_tensor_scan linear recurrences
            (i' = 0.8 i' + J, V = 0.9 V + i', where V = 10*v_li).
  LIF k>=1: per-timestep scalar_tensor_tensor ops with scaled states
            P = 100*v, Q = 10*i so all cross terms have unit coefficient:
            P = 0.9P + Q; spike = P > 100; P = (P<=100)*P; Q = 0.8Q + V[t].
  early exit: the reset-free LIF1 membrane is a triple first-order filter of
            V0 with kernel l1-norm <= 50, so 50*max|V0| < 95 (< threshold 100)
            proves layer 1 never spikes, hence layers 2..5 are exactly zero ->
            output the zero logit tile. Anything near threshold takes the
            exact slow path (runtime If). Layer-0 matmul runs fp8e4m3
            DoubleRow (weights host-scaled x64; drains rescale by 1/64).
  output:   max over t of V5/10, log_softmax on host (tiny [128,10]).

Internal dtypes: bf16 states/spikes/weights, fp32 scan states + PSUM.
"""

import os
import sys
import numpy as np
import ml_dtypes

import concourse.bass as bass
import concourse.tile as tile
from concourse import bacc, mybir
from concourse.bass_utils import run_bass_kernel_spmd

AL = mybir.AluOpType
F32 = mybir.dt.float32
BF16 = mybir.dt.bfloat16
FP8 = mybir.dt.float8e4
FP8_NP = mybir.dt.np(mybir.dt.float8e4)
W0_SCALE = 64.0

N_CORES = 8
B_CORE = 16
T = 32

LAYER_SIZES = [(2000, 12000), (1500, 2000), (1000, 1500), (500, 1000), (100, 500), (10, 100)]
IN_PAD = [12032, 2048, 1536, 1024, 512, 128]
OUT_PAD = [2048, 1536, 1024, 512, 128, 16]
IC = [94, 16, 12, 8, 4, 1]      # input chunks of 128 (contraction)
MC = [16, 12, 8, 4, 1, 1]       # output chunks (M tiles)
M_SIZE = [128, 128, 128, 128, 128, 16]
G0_RANGES = [(0, 12), (12, 48), (48, 94)]  # stage-0 chunk groups

LAST_EXEC_TIME_NS = None

DT_DECAY_V = np.float32(0.1)   # DT*TAU_MEM_INV
V_TH = np.float32(1.0)


def _enc_first_spike_step(x_scalar):
    """fp32 encoder sim (exactly mirrors reference arithmetic); first spike step or None."""
    f32 = np.float32
    v = f32(0.0)
    x = f32(x_scalar)
    for t in range(T):
        v = f32(v + f32(DT_DECAY_V * f32(-v + x)))
        if f32(v - V_TH) > 0:
            return t
    return None


def _stage0_tables():
    """Host-precomputed structure of the encoder+LIF0 cascade.

    The encoder resets to exactly 0 on spike, so its spike train is periodic
    with period p(x) = 1 + first_spike_step(x); LIF0's response to a period-p
    train is a fixed pattern G[t, p].  The map x -> LIF0-spike-train is
    piecewise constant in x; we compress it to the breakpoints where the
    pattern actually changes and pack patterns as integer codes.
    Returns (breaks [(B_n, delta_n)...], bit_ts [t for each bit, ascending]).
    """
    f32 = np.float32
    # G[t, c]: c = 0 -> silent input; c = p -> period p
    G = np.zeros((T, 34), np.int64)
    for c in range(1, 33):
        v = f32(0.0)
        i = f32(0.0)
        for t in range(T):
            inp = f32(1.0) if (t + 1) % c == 0 else f32(0.0)
            v_dec = f32(v + f32(DT_DECAY_V * f32(-v + i)))
            i_dec = f32(i * f32(0.8))
            z = 1 if f32(v_dec - V_TH) > 0 else 0
            v = f32(0.0) if z else v_dec
            i = f32(i_dec + inp)
            G[t, c] = z
    bit_ts = [t for t in range(T) if G[t].any()]
    code = {c: sum(int(G[ts, c]) << j for j, ts in enumerate(bit_ts)) for c in range(34)}
    code[33] = 0  # period > 32 == silent
    used = [n for n in range(1, 33) if code[n] != code[n + 1]]

    # fp32-exact breakpoints: B_n = min x with first_spike_step <= n-1
    breaks = []
    for n in used:
        lo = np.float32(1.0).view(np.int32)
        hi = np.float32(20.0).view(np.int32)
        while int(hi) - int(lo) > 1:
            mid = np.int32((int(lo) + int(hi)) // 2)
            s = _enc_first_spike_step(mid.view(np.float32))
            if s is not None and s <= n - 1:
                hi = mid
            else:
                lo = mid
        breaks.append((float(np.int32(hi).view(np.float32)), float(code[n] - code[n + 1])))
    return breaks, bit_ts


def _install_ntff_hook():
    import types
    if "antenv.axon_hooks" in sys.modules:
        return
    try:
        mod = types.ModuleType("antenv.axon_hooks")
        mod._hook = None
        mod.set_axon_ntff_profile_hook = lambda h: setattr(mod, "_hook", h)
        mod.get_axon_ntff_profile_hook = lambda: mod._hook
        sys.modules["antenv.axon_hooks"] = mod
        from trn_agent_boot.trn_boot import _ntff_profile_via_ctypes
        mod._hook = _ntff_profile_via_ctypes("/opt/axon/libaxon_pjrt.so")
    except Exception:
        pass


def build_body(tc, ctx, nc, xs_ap, w_aps, out_ap, taps=None):
    from contextlib import ExitStack

    const = ctx.enter_context(tc.tile_pool(name="const", bufs=1))
    psum = ctx.enter_context(tc.tile_pool(name="psum", bufs=8, space="PSUM"))
    ijpool = ctx.enter_context(tc.tile_pool(name="ij", bufs=2))
    spool = ctx.enter_context(tc.tile_pool(name="spikes", bufs=2))

    mask08 = const.tile([128, 512], BF16)
    mask09 = const.tile([128, 512], BF16)

    def emit_masks():
        # decay masks with 0.0 at t=0 of each batch segment (scan segmentation)
        nc.vector.memset(mask08[:], 0.8)
        nc.vector.memset(mask08[:].rearrange("p (b t) -> p b t", b=B_CORE)[:, :, 0:1], 0.0)
        nc.vector.memset(mask09[:], 0.9)
        nc.vector.memset(mask09[:].rearrange("p (b t) -> p b t", b=B_CORE)[:, :, 0:1], 0.0)

    Jsb = const.tile([128, MC[0], 512], BF16)  # layer-0 spilled J accumulator

    spikes = None  # current layer's input spike tensor, [128, IC[k], 16, 32] bf16

    with ExitStack() as phase0:
        p0 = phase0.enter_context(tc.tile_pool(name="phase0", bufs=1))
        w0pool = phase0.enter_context(tc.tile_pool(name="w0s", bufs=2))

        breaks, bit_ts = _stage0_tables()
        nbits = len(bit_ts)
        assert nbits <= 24, "spike code must fit fp32 integer range"

        xr_sb = p0.tile([128, 94, B_CORE], F32)
        nc.sync.dma_start(xr_sb[:], xs_ap)

        # S0 layout [p, chunk, t, b]: per-t spike writes hit contiguous 16-elem
        # runs; matmul rhs columns come out (t, b)-ordered (fixed in the drain).
        S0 = p0.tile([128, 94, T, B_CORE], FP8)
        W = p0.tile([128, 94, B_CORE], F32)
        tmp = p0.tile([128, 94, B_CORE], F32)

        for g, (c0, c1) in enumerate(G0_RANGES):
            # ---- stage-0 for this chunk group: build spike-codes, extract bits ----
            sW = W[:, c0:c1, :]
            stmp = tmp[:, c0:c1, :]
            sxr = xr_sb[:, c0:c1, :]
            if g < 2:
                nc.vector.memset(S0[:, c0:c1, :, :], 0.0)
            else:
                nc.gpsimd.memset(S0[:, c0:c1, :, :], 0.0)
            for i, (bn, dn) in enumerate(breaks):
                nc.vector.tensor_scalar(stmp, sxr, bn, dn, AL.is_ge, AL.mult)
                if i == 0:
                    nc.vector.tensor_copy(sW, stmp)
                else:
                    nc.vector.tensor_tensor(sW, sW, stmp, AL.add)
            for j in range(nbits - 1, -1, -1):
                nc.vector.tensor_scalar(
                    S0[:, c0:c1, bit_ts[j], :], sW, float(1 << j), None, AL.is_ge
                )
                if j > 0:
                    # W -= (W >= 2^j) * 2^j  (strip the extracted top bit)
                    nc.vector.tensor_scalar(
                        stmp, sW, float(1 << j), float(1 << j), AL.is_ge, AL.mult
                    )
                    nc.vector.tensor_tensor(sW, sW, stmp, AL.subtract)
            # ---- layer-0 matmul for this chunk group (fp8 DoubleRow, k-pairs) ----
            p0r, p1r = c0 // 2, c1 // 2
            for mp in range(8):
                wt = w0pool.tile([128, p1r - p0r, 2, 256], FP8)
                nc.sync.dma_start(wt[:], w_aps[0][mp, :, p0r:p1r, :, :])
                for half in range(2):
                    m = mp * 2 + half
                    ps = psum.tile([128, 512], F32)
                    for kp in range(p0r, p1r):
                        nc.tensor.matmul(
                            ps[:],
                            wt[:, kp - p0r, :, half * 128:(half + 1) * 128],
                            S0[:, 2 * kp:2 * kp + 2, :, :],
                            start=(kp == p0r),
                            stop=(kp == p1r - 1),
                            perf_mode=mybir.MatmulPerfMode.DoubleRow,
                        )
                    # drain PSUM -> Jsb: ACT (PSUM-proximate, otherwise idle) does
                    # the scaled (t,b)->(b,t) permuting copy; DVE only adds bf16.
                    ps_bt = ps[:].rearrange("p (t b) -> p b t", t=T)
                    j_bt = Jsb[:, m, :].rearrange("p (b t) -> p b t", b=B_CORE)
                    use_act = (m % 2 == 0)  # split drain load between ACT and DVE
                    if g == 0:
                        if use_act:
                            nc.scalar.activation(
                                j_bt, ps_bt, mybir.ActivationFunctionType.Copy,
                                scale=1.0 / W0_SCALE,
                            )
                        else:
                            nc.vector.tensor_scalar(
                                j_bt, ps_bt, 1.0 / W0_SCALE, None, AL.mult
                            )
                    elif use_act:
                        stg = w0pool.tile([128, 512], BF16, tag="stg")
                        nc.scalar.activation(
                            stg[:].rearrange("p (b t) -> p b t", b=B_CORE), ps_bt,
                            mybir.ActivationFunctionType.Copy, scale=1.0 / W0_SCALE,
                        )
                        nc.vector.tensor_tensor(Jsb[:, m, :], Jsb[:, m, :], stg[:], AL.add)
                    else:
                        nc.vector.scalar_tensor_tensor(
                            j_bt, ps_bt, 1.0 / W0_SCALE, j_bt, AL.mult, AL.add
                        )

    # ---- per layer: scans (LI cell) -> LIF -> next matmul ----
    mx = const.tile([128, MC[0]], F32)  # per-m-chunk max of the LIF1 bound

    def lif_phase(k, V, pk):
        nonlocal spikes
        C = MC[k]
        Vv = V[:].rearrange("p m (b t) -> p m b t", t=T)
        S = spool.tile([128, C, B_CORE, T], BF16, tag="S")
        P = pk.tile([128, C, B_CORE], BF16, tag="P")
        Q = pk.tile([128, C, B_CORE], BF16, tag="Q")
        nc.vector.memset(P[:], 0.0)
        nc.vector.memset(Q[:], 0.0)
        for t in range(T):
            nc.vector.scalar_tensor_tensor(P[:], P[:], 0.9, Q[:], AL.mult, AL.add)
            nc.vector.tensor_scalar(S[:, :, :, t], P[:], 100.0, None, AL.is_gt)
            nc.vector.scalar_tensor_tensor(P[:], P[:], 100.0, P[:], AL.is_le, AL.mult)
            nc.vector.scalar_tensor_tensor(Q[:], Q[:], 0.8, Vv[:, :, :, t], AL.mult, AL.add)
        spikes = S

    def layer_phase(k):
        nonlocal spikes
        M = M_SIZE[k]
        with ExitStack() as ph:
            pk = ph.enter_context(tc.tile_pool(name=f"phase{k + 1}", bufs=1))
            if k == 5:
                V = pk.tile([M, 512], F32, tag="V5")
            elif k == 0:
                V = const.tile([128, MC[k], 512], BF16)  # outlives the phase (Else reads it)
            else:
                V = pk.tile([128, MC[k], 512], BF16, tag=f"V{k}")

            if k >= 1:
                wk_sb = pk.tile([128, IC[k], OUT_PAD[k]], BF16, tag=f"w{k}")
                nc.sync.dma_start(wk_sb[:], w_aps[k])

            for m in range(MC[k]):
                if k == 0:
                    j_src = Jsb[:, m, :]
                else:
                    ps = psum.tile([128, 512], F32)
                    for kc in range(IC[k]):
                        nc.tensor.matmul(
                            ps[:M, :],
                            wk_sb[:, kc, m * 128:m * 128 + M],
                            spikes[:, kc, :, :],
                            start=(kc == 0),
                            stop=(kc == IC[k] - 1),
                        )
                    j_src = ps[:M, :]
                ij = ijpool.tile([128, 512], BF16)
                nc.vector.tensor_tensor_scan(ij[:M, :], mask08[:M, :], j_src, 0.0, AL.mult, AL.add)
                if k == 5:
                    nc.vector.tensor_tensor_scan(V[:, :], mask09[:M, :], ij[:M, :], 0.0, AL.mult, AL.add)
                else:
                    nc.vector.tensor_tensor_scan(V[:, m, :], mask09[:, :], ij[:, :], 0.0, AL.mult, AL.add)
                if k == 0:
                    # LIF1 membrane bound: the reset-free membrane is
                    # scan(0.9, scan(0.8, V)) whose kernel has l1-norm <= 50,
                    # so 50*max|V| < 95 (< threshold 100) proves layer 1
                    # never spikes. Conservative; failures take the slow path.
                    nc.vector.tensor_reduce(
                        mx[:, m:m + 1], V[:, m, :], mybir.AxisListType.X, AL.max,
                        apply_absolute_value=True,
                    )

            if taps is not None and k in taps:
                nc.sync.dma_start(taps[k], V[:] if k == 5 else V[:, :, :])

            if k == 5:
                rmax = pk.tile([M, B_CORE], F32)
                nc.vector.tensor_reduce(
                    rmax[:], V[:].rearrange("p (b t) -> p b t", b=B_CORE),
                    mybir.AxisListType.X, AL.max,
                )
                nc.sync.dma_start(out_ap, rmax[:])
            elif k >= 1:
                lif_phase(k, V, pk)
        return V

    emit_masks()
    V0 = layer_phase(0)

    # ---- early exit: if the LIF1 membrane bound never nears threshold, layer 1
    # cannot spike, hence layers 2..5 are exactly zero (J=0 -> V=0 -> no spikes
    # inductively) and the output is the all-zero logit tile. Conservative
    # threshold 95 < 100 routes anything near threshold to the exact slow path.
    from concourse import bass_isa
    amax = const.tile([128, 1], F32)
    nc.vector.tensor_reduce(amax[:], mx[:, :], mybir.AxisListType.X, AL.max)
    gmax = const.tile([128, 1], F32)
    nc.gpsimd.partition_all_reduce(gmax[:], amax[:], 128, bass_isa.ReduceOp.max)
    gmax_s = const.tile([1, 1], F32)
    nc.vector.tensor_scalar(gmax_s[:], gmax[0:1, 0:1], 50.0, None, AL.mult)
    gmax_i = const.tile([1, 1], mybir.dt.int32)
    nc.vector.tensor_copy(gmax_i[:], gmax_s[:])
    _, (sval,) = nc.values_load_multi_w_load_instructions(
        gmax_i[0:1, 0:1], skip_runtime_bounds_check=True
    )
    with tc.If(sval < 95) as cmp:  # gmax_i is the value-cast (truncated) fp32 max
        zero_out = const.tile([M_SIZE[5], B_CORE], F32)
        nc.vector.memset(zero_out[:], 0.0)
        nc.sync.dma_start(out_ap, zero_out[:])
    with cmp.Else():
        with ExitStack() as phl:
            pl = phl.enter_context(tc.tile_pool(name="lif1", bufs=1))
            lif_phase(0, V0, pl)
        for k in range(1, 6):
            layer_phase(k)


def build_nc(taps_spec=None):
    from contextlib import ExitStack

    nc = bacc.Bacc("TRN2", debug=False, num_devices=N_CORES)
    xs = nc.dram_tensor("xs", [128, 94, B_CORE], F32, kind="ExternalInput")
    w_t = [nc.dram_tensor("w0t", [8, 128, 47, 2, 256], FP8, kind="ExternalInput")]
    for k in range(1, 6):
        w_t.append(
            nc.dram_tensor(f"w{k}t", [128, IC[k], OUT_PAD[k]], BF16, kind="ExternalInput")
        )
    out = nc.dram_tensor("out", [M_SIZE[5], B_CORE], F32, kind="ExternalOutput")

    taps = None
    if taps_spec:
        taps = {}
        for k in taps_spec:
            if k == 5:
                th = nc.dram_tensor(f"tapV{k}", [M_SIZE[5], 512], F32, kind="ExternalOutput")
                taps[k] = th.ap()
            else:
                th = nc.dram_tensor(f"tapV{k}", [128, MC[k], 512], BF16, kind="ExternalOutput")
                taps[k] = th.ap()

    with tile.TileContext(nc) as tc, ExitStack() as ctx:
        build_body(tc, ctx, nc, xs.ap(), [w.ap() for w in w_t], out.ap(), taps=taps)
    nc.compile()
    return nc


def prep_inputs(images, ws):
    """Host-side marshalling: pad/transpose/cast weights, rearrange images."""
    x = np.asarray(images).reshape(128, -1).astype(np.float32)  # [B, 12000]
    xs = np.zeros((128, 12032), np.float32)
    xs[:, :12000] = x
    # [p, chunk, b] with feature f = chunk*128 + p
    xs_r = xs.reshape(128, 94, 128).transpose(2, 1, 0)  # [128p, 94c, 128b]
    xs_cores = [
        np.ascontiguousarray(xs_r[:, :, c * B_CORE:(c + 1) * B_CORE])
        for c in range(N_CORES)
    ]

    w_prepped = []
    wT0 = np.zeros((12032, 2048), np.float32)
    wT0[:12000, :2000] = np.asarray(ws[0]).T * np.float32(W0_SCALE)
    # [8 mp, 128 p, 47 kcp, 2 j, 256 m]: feature f = (2*kcp + j)*128 + p
    w0p = wT0.reshape(47, 2, 128, 8, 256).transpose(3, 2, 0, 1, 4)
    w_prepped.append(np.ascontiguousarray(w0p.astype(FP8_NP)))
    for k in range(1, 6):
        out_f, in_f = LAYER_SIZES[k]
        wTk = np.zeros((IN_PAD[k], OUT_PAD[k]), np.float32)
        wTk[:in_f, :out_f] = np.asarray(ws[k]).T
        wkp = wTk.reshape(IC[k], 128, OUT_PAD[k]).transpose(1, 0, 2)  # [128p, IC, OUT]
        w_prepped.append(np.ascontiguousarray(wkp.astype(ml_dtypes.bfloat16)))
    return xs_cores, w_prepped


_NC_CACHE = {}


def kernel(images, w0, w1, w2, w3, w4, w5):
    global LAST_EXEC_TIME_NS
    ws = [w0, w1, w2, w3, w4, w5]
    xs_cores, w_prepped = prep_inputs(images, ws)

    trace = os.environ.get("KERNEL_TRACE", "0") == "1"
    if trace:
        _install_ntff_hook()

    if "nc" not in _NC_CACHE:
        _NC_CACHE["nc"] = build_nc()
    nc = _NC_CACHE["nc"]

    in_maps = []
    for c in range(N_CORES):
        m = {"xs": xs_cores[c], "w0t": w_prepped[0]}
        for k in range(1, 6):
            m[f"w{k}t"] = w_prepped[k]
        in_maps.append(m)

    res = run_bass_kernel_spmd(
        nc, in_maps, core_ids=list(range(N_CORES)), trace=trace
    )
    LAST_EXEC_TIME_NS = res.exec_time_ns
    _NC_CACHE["res"] = res

    # out[c] is [16 feats, 16 batch]; valid feats :10; logits = max_t(V5)/10
    logits = np.concatenate(
        [np.asarray(res.results[c]["out"])[:10, :].T for c in range(N_CORES)], axis=0
    ).astype(np.float32) / np.float32(10.0)
    mx = logits.max(axis=1, keepdims=True)
    sh = logits - mx
    out = sh - np.log(np.exp(sh).sum(axis=1, keepdims=True))
    return out.astype(np.float32)



# revision 9
# speedup vs baseline: 1.7810x; 1.7810x over previous
"""Trainium2 Bass kernel for nn_CaptchaRecognizer (norse-style SNN).

Strategy (pure data-parallel over batch, 8 NeuronCores, 16 images each):

The encoder resets to exactly 0 on spike, so the encoder+LIF0 cascade is a
piecewise-constant function of x alone: only 4 fp32-exact breakpoints B_n
matter, and the LIF0 spike train is EXACTLY LINEAR in the 4 nested threshold
masks u_n = (x >= B_n):   z0[t] = sum_n D[n,t] * u_n   (D host-precomputed).

Hence the layer-0 LI membrane is   V0[t,b,o] = sum_n H[n,t] * Y_n[b,o]   with
Y_n = u_n @ w0^T and H = LI-filtered D.  The 32-timestep spike matmul of the
reference collapses to a 4-channel mask matmul: out rows (b,i) = 16*4 = 64
instead of t*b = 512 — 8x fewer MACs, one stream of w0 from HBM (fp8 x64,
DoubleRow, K-accumulated in PSUM; stationary = masks, moving = w0 columns).

  early exit: the reset-free LIF1 membrane is a triple first-order filter of
            V0 with kernel l1-norm <= 50. A cheap certified bound
            max|V0| <= sum_n (max_t|H_n|) |Y_n|  (PE reduction over the 4
            channels) gives 50*bound < 95 (< threshold 100) => layer 1 never
            spikes => layers 2..5 exactly zero => output the zero logit tile.
  slow path: runtime If; V0 materialized exactly from Y by a tiny PE
            expansion against H, then the original per-layer pipeline
            (LIF via scalar_tensor_tensor steps, LI via tensor_tensor_scan
            linear recurrences, bf16 matmuls for w1..w5).
  output:   max over t of V5/10, log_softmax on host (tiny [128,10]).

Internal dtypes: fp8 masks/w0 (x64), bf16 states/Y/weights, fp32 PSUM.
"""

import os
import sys
import numpy as np
import ml_dtypes

import concourse.bass as bass
import concourse.tile as tile
from concourse import bacc, mybir
from concourse.bass_utils import run_bass_kernel_spmd

AL = mybir.AluOpType
F32 = mybir.dt.float32
BF16 = mybir.dt.bfloat16
FP8 = mybir.dt.float8e4
FP8_NP = mybir.dt.np(mybir.dt.float8e4)
BF16_NP = ml_dtypes.bfloat16
W0_SCALE = 64.0

N_CORES = 8
B_CORE = 16
T = 32
NMASK = 4

LAYER_SIZES = [(2000, 12000), (1500, 2000), (1000, 1500), (500, 1000), (100, 500), (10, 100)]
IN_PAD = [12032, 2048, 1536, 1024, 512, 128]
OUT_PAD = [2048, 1536, 1024, 512, 128, 16]
IC = [94, 16, 12, 8, 4, 1]      # input chunks of 128 (contraction)
MC = [16, 12, 8, 4, 1, 1]       # output chunks (M tiles)
M_SIZE = [128, 128, 128, 128, 128, 16]
KP0 = 47                         # layer-0 DoubleRow k-pairs

LAST_EXEC_TIME_NS = None

DT_DECAY_V = np.float32(0.1)   # DT*TAU_MEM_INV
V_TH = np.float32(1.0)


def _enc_first_spike_step(x_scalar):
    """fp32 encoder sim (exactly mirrors reference arithmetic); first spike step or None."""
    f32 = np.float32
    v = f32(0.0)
    x = f32(x_scalar)
    for t in range(T):
        v = f32(v + f32(DT_DECAY_V * f32(-v + x)))
        if f32(v - V_TH) > 0:
            return t
    return None


def _stage0_tables():
    """Host-precomputed structure of the encoder+LIF0 cascade.

    The encoder resets to exactly 0 on spike, so its spike train is periodic
    with period p(x) = 1 + first_spike_step(x); LIF0's response to a period-p
    train is a fixed pattern G[t, p].  The map x -> LIF0-spike-train is
    piecewise constant in x; we compress it to the breakpoints where the
    pattern actually changes and pack patterns as integer codes.
    Returns (breaks [(B_n, delta_n)...], bit_ts [t for each bit, ascending]).
    """
    f32 = np.float32
    # G[t, c]: c = 0 -> silent input; c = p -> period p
    G = np.zeros((T, 34), np.int64)
    for c in range(1, 33):
        v = f32(0.0)
        i = f32(0.0)
        for t in range(T):
            inp = f32(1.0) if (t + 1) % c == 0 else f32(0.0)
            v_dec = f32(v + f32(DT_DECAY_V * f32(-v + i)))
            i_dec = f32(i * f32(0.8))
            z = 1 if f32(v_dec - V_TH) > 0 else 0
            v = f32(0.0) if z else v_dec
            i = f32(i_dec + inp)
            G[t, c] = z
    bit_ts = [t for t in range(T) if G[t].any()]
    code = {c: sum(int(G[ts, c]) << j for j, ts in enumerate(bit_ts)) for c in range(34)}
    code[33] = 0  # period > 32 == silent
    used = [n for n in range(1, 33) if code[n] != code[n + 1]]

    # fp32-exact breakpoints: B_n = min x with first_spike_step <= n-1
    breaks = []
    for n in used:
        lo = np.float32(1.0).view(np.int32)
        hi = np.float32(20.0).view(np.int32)
        while int(hi) - int(lo) > 1:
            mid = np.int32((int(lo) + int(hi)) // 2)
            s = _enc_first_spike_step(mid.view(np.float32))
            if s is not None and s <= n - 1:
                hi = mid
            else:
                lo = mid
        breaks.append((float(np.int32(hi).view(np.float32)), float(code[n] - code[n + 1])))
    return breaks, bit_ts


def _mask_tables():
    """Per-breakpoint spike-train deltas D [4, T] and LI-filtered H [4, T].

    z0[t] (LIF0 spikes) = sum_n (x >= B_n) * D[n, t]  exactly (nested masks).
    V0[t] (scaled LI0 membrane, V = 10*v) = sum_n H[n, t] * Y_n, with
    H the (i' = 0.8 i' + D; V = 0.9 V + i') double filter of D.
    """
    breaks, bit_ts = _stage0_tables()
    assert len(breaks) == NMASK
    deltas = [d for (_, d) in breaks]
    Bs = [b for (b, _) in breaks]
    # Bs descending: passing B_n implies passing all later (smaller) breakpoints.
    csum = np.cumsum(deltas[::-1])[::-1]  # code when masks n..3 are on

    def bits(c):
        c = int(round(c))
        return np.array([(c >> j) & 1 for j in range(len(bit_ts))], np.float64)

    pats = [bits(c) for c in csum] + [np.zeros(len(bit_ts))]
    D = np.zeros((NMASK, T))
    for n in range(NMASK):
        dv = pats[n] - pats[n + 1]
        for j, t in enumerate(bit_ts):
            D[n, t] = dv[j]
    H = np.zeros((NMASK, T))
    for n in range(NMASK):
        ip = 0.0
        V = 0.0
        for t in range(T):
            ip = 0.8 * ip + D[n, t]
            V = 0.9 * V + ip
            H[n, t] = V
    return Bs, D, H


def _install_ntff_hook():
    import types
    if "antenv.axon_hooks" in sys.modules:
        return
    try:
        mod = types.ModuleType("antenv.axon_hooks")
        mod._hook = None
        mod.set_axon_ntff_profile_hook = lambda h: setattr(mod, "_hook", h)
        mod.get_axon_ntff_profile_hook = lambda: mod._hook
        sys.modules["antenv.axon_hooks"] = mod
        from trn_agent_boot.trn_boot import _ntff_profile_via_ctypes
        mod._hook = _ntff_profile_via_ctypes("/opt/axon/libaxon_pjrt.so")
    except Exception:
        pass


def build_body(tc, ctx, nc, xs_ap, w_aps, h_ap, a_ap, out_ap):
    from contextlib import ExitStack

    Bs, _D, _H = _mask_tables()

    const = ctx.enter_context(tc.tile_pool(name="const", bufs=1))
    psum = ctx.enter_context(tc.tile_pool(name="psum", bufs=8, space="PSUM"))
    ijpool = ctx.enter_context(tc.tile_pool(name="ij", bufs=2))
    spool = ctx.enter_context(tc.tile_pool(name="spikes", bufs=2))

    mask08 = const.tile([128, 512], BF16)
    mask09 = const.tile([128, 512], BF16)

    def emit_masks():
        # decay masks with 0.0 at t=0 of each batch segment (scan segmentation)
        nc.vector.memset(mask08[:], 0.8)
        nc.vector.memset(mask08[:].rearrange("p (b t) -> p b t", b=B_CORE)[:, :, 0:1], 0.0)
        nc.vector.memset(mask09[:], 0.9)
        nc.vector.memset(mask09[:].rearrange("p (b t) -> p b t", b=B_CORE)[:, :, 0:1], 0.0)

    # ---- fast path: masks -> Y = u @ w0^T (64 rows) -> certified LIF1 bound ----
    xr_sb = const.tile([128, KP0, 2, B_CORE], F32)
    nc.sync.dma_start(xr_sb[:], xs_ap)

    # u layout [p, kp, j, (b,i)] b-major: stationary operand of the DR matmul
    u = const.tile([128, KP0, 2, B_CORE, NMASK], FP8)
    for i, bn in enumerate(Bs):
        nc.vector.tensor_scalar(u[:, :, :, :, i], xr_sb[:], float(bn), None, AL.is_ge)

    hc = const.tile([NMASK, T], BF16)      # H for the Else expansion
    nc.sync.dma_start(hc[:], h_ap)
    ac = const.tile([B_CORE * NMASK, B_CORE], BF16)  # bound-reduction matrix
    nc.sync.dma_start(ac[:], a_ap)

    Ysb = const.tile([B_CORE * NMASK, 4, 512], BF16)    # descaled Y, for Else
    absY = const.tile([B_CORE * NMASK, 4, 512], BF16)   # |Y| for the bound

    with ExitStack() as phase0:
        w0pool = phase0.enter_context(tc.tile_pool(name="w0s", bufs=3))
        ps = [psum.tile([B_CORE * NMASK, 512], F32, name=f"ps{og}", bufs=1) for og in range(4)]
        for kp in range(KP0):
            wt = w0pool.tile([128, 2, 2048], FP8)
            nc.sync.dma_start(wt[:], w_aps[0][kp])
            for og in range(4):
                nc.tensor.matmul(
                    ps[og][:],
                    u[:, kp, :, :, :],
                    wt[:, :, og * 512:(og + 1) * 512],
                    start=(kp == 0),
                    stop=(kp == KP0 - 1),
                    perf_mode=mybir.MatmulPerfMode.DoubleRow,
                )
        for og in range(4):
            nc.vector.tensor_scalar(
                Ysb[:, og, :], ps[og][:], 1.0 / W0_SCALE, None, AL.mult
            )
            nc.scalar.activation(
                absY[:, og, :], ps[og][:], mybir.ActivationFunctionType.Abs,
                scale=1.0 / W0_SCALE,
            )
        # R[b, o] = sum_i A_i |Y_i[b, o]| via PE: stationary = A [(b,i), b'], then
        # 50 * max R < 95 certifies "layer 1 never spikes".
        rmx = const.tile([B_CORE, 4], F32)
        for og in range(4):
            psr = psum.tile([B_CORE, 512], F32, bufs=1)
            nc.tensor.matmul(psr[:], ac[:], absY[:, og, :], start=True, stop=True)
            nc.vector.tensor_reduce(rmx[:, og:og + 1], psr[:], mybir.AxisListType.X, AL.max)

    from concourse import bass_isa
    amax = const.tile([128, 1], F32)
    nc.vector.memset(amax[:], 0.0)
    nc.vector.tensor_reduce(amax[0:B_CORE, :], rmx[:], mybir.AxisListType.X, AL.max)
    gmax = const.tile([128, 1], F32)
    nc.gpsimd.partition_all_reduce(gmax[:], amax[:], 128, bass_isa.ReduceOp.max)
    gmax_s = const.tile([1, 1], F32)
    nc.vector.tensor_scalar(gmax_s[:], gmax[0:1, 0:1], 50.0, None, AL.mult)
    gmax_i = const.tile([1, 1], mybir.dt.int32)
    nc.vector.tensor_copy(gmax_i[:], gmax_s[:])
    _, (sval,) = nc.values_load_multi_w_load_instructions(
        gmax_i[0:1, 0:1], skip_runtime_bounds_check=True
    )

    # ---- slow-path helpers (baseline per-layer pipeline) ----
    spikes = None  # current layer's input spike tensor, [128, IC[k], 16, 32] bf16

    def lif_phase(k, V, pk):
        nonlocal spikes
        C = MC[k]
        Vv = V[:].rearrange("p m (b t) -> p m b t", t=T)
        S = spool.tile([128, C, B_CORE, T], BF16, tag="S")
        P = pk.tile([128, C, B_CORE], BF16, tag="P")
        Q = pk.tile([128, C, B_CORE], BF16, tag="Q")
        nc.vector.memset(P[:], 0.0)
        nc.vector.memset(Q[:], 0.0)
        for t in range(T):
            nc.vector.scalar_tensor_tensor(P[:], P[:], 0.9, Q[:], AL.mult, AL.add)
            nc.vector.tensor_scalar(S[:, :, :, t], P[:], 100.0, None, AL.is_gt)
            nc.vector.scalar_tensor_tensor(P[:], P[:], 100.0, P[:], AL.is_le, AL.mult)
            nc.vector.scalar_tensor_tensor(Q[:], Q[:], 0.8, Vv[:, :, :, t], AL.mult, AL.add)
        spikes = S

    def layer_phase(k):
        nonlocal spikes
        M = M_SIZE[k]
        with ExitStack() as ph:
            pk = ph.enter_context(tc.tile_pool(name=f"phase{k + 1}", bufs=1))
            if k == 5:
                V = pk.tile([M, 512], F32, tag="V5")
            else:
                V = pk.tile([128, MC[k], 512], BF16, tag=f"V{k}")

            wk_sb = pk.tile([128, IC[k], OUT_PAD[k]], BF16, tag=f"w{k}")
            nc.sync.dma_start(wk_sb[:], w_aps[k])

            for m in range(MC[k]):
                ps = psum.tile([128, 512], F32, bufs=2)
                for kc in range(IC[k]):
                    nc.tensor.matmul(
                        ps[:M, :],
                        wk_sb[:, kc, m * 128:m * 128 + M],
                        spikes[:, kc, :, :],
                        start=(kc == 0),
                        stop=(kc == IC[k] - 1),
                    )
                j_src = ps[:M, :]
                ij = ijpool.tile([128, 512], BF16)
                nc.vector.tensor_tensor_scan(ij[:M, :], mask08[:M, :], j_src, 0.0, AL.mult, AL.add)
                if k == 5:
                    nc.vector.tensor_tensor_scan(V[:, :], mask09[:M, :], ij[:M, :], 0.0, AL.mult, AL.add)
                else:
                    nc.vector.tensor_tensor_scan(V[:, m, :], mask09[:, :], ij[:, :], 0.0, AL.mult, AL.add)

            if k == 5:
                rmax = pk.tile([M, B_CORE], F32)
                nc.vector.tensor_reduce(
                    rmax[:], V[:].rearrange("p (b t) -> p b t", b=B_CORE),
                    mybir.AxisListType.X, AL.max,
                )
                nc.sync.dma_start(out_ap, rmax[:])
            else:
                lif_phase(k, V, pk)

    with tc.If(sval < 95) as cmp:
        zero_out = const.tile([M_SIZE[5], B_CORE], F32)
        nc.vector.memset(zero_out[:], 0.0)
        nc.sync.dma_start(out_ap, zero_out[:])
    with cmp.Else():
        emit_masks()
        with ExitStack() as phl:
            pl = phl.enter_context(tc.tile_pool(name="lif1", bufs=1))
            # transpose Y to partitions = i for PE expansion against H
            Yt = pl.tile([NMASK, B_CORE, 4, 512], BF16, tag="Yt")
            for b in range(B_CORE):
                nc.sync.dma_start(
                    Yt[:, b, :, :], Ysb[b * NMASK:(b + 1) * NMASK, :, :]
                )
            V0 = pl.tile([128, MC[0], 512], BF16, tag="V0")
            for m in range(MC[0]):
                psv = psum.tile([128, 512], F32, bufs=1)
                for b in range(B_CORE):
                    nc.tensor.matmul(
                        psv[:, b * T:(b + 1) * T],
                        Yt[:, b, m // 4, (m % 4) * 128:(m % 4) * 128 + 128],
                        hc[:],
                        start=True, stop=True,
                    )
                nc.scalar.activation(
                    V0[:, m, :], psv[:], mybir.ActivationFunctionType.Copy, scale=1.0
                )
            lif_phase(0, V0, pl)
        for k in range(1, 6):
            layer_phase(k)


def build_nc():
    from contextlib import ExitStack

    nc = bacc.Bacc("TRN2", debug=False, num_devices=N_CORES)
    xs = nc.dram_tensor("xs", [128, KP0, 2, B_CORE], F32, kind="ExternalInput")
    w_t = [nc.dram_tensor("w0t", [KP0, 128, 2, 2048], FP8, kind="ExternalInput")]
    for k in range(1, 6):
        w_t.append(
            nc.dram_tensor(f"w{k}t", [128, IC[k], OUT_PAD[k]], BF16, kind="ExternalInput")
        )
    hconst = nc.dram_tensor("hconst", [NMASK, T], BF16, kind="ExternalInput")
    aconst = nc.dram_tensor("aconst", [B_CORE * NMASK, B_CORE], BF16, kind="ExternalInput")
    out = nc.dram_tensor("out", [M_SIZE[5], B_CORE], F32, kind="ExternalOutput")

    with tile.TileContext(nc) as tc, ExitStack() as ctx:
        build_body(tc, ctx, nc, xs.ap(), [w.ap() for w in w_t],
                   hconst.ap(), aconst.ap(), out.ap())
    nc.compile()
    return nc


def prep_inputs(images, ws):
    """Host-side marshalling: pad/transpose/cast weights, rearrange images."""
    x = np.asarray(images).reshape(128, -1).astype(np.float32)  # [B, 12000]
    xs = np.zeros((128, 12032), np.float32)
    xs[:, :12000] = x
    # [p, kp, j, b] with feature f = (2*kp + j)*128 + p
    xs_r = xs.reshape(128, 47, 2, 128).transpose(3, 1, 2, 0)  # [128p, 47, 2, 128b]
    xs_cores = [
        np.ascontiguousarray(xs_r[:, :, :, c * B_CORE:(c + 1) * B_CORE])
        for c in range(N_CORES)
    ]

    w_prepped = []
    wT0 = np.zeros((12032, 2048), np.float32)
    wT0[:12000, :2000] = np.asarray(ws[0]).T * np.float32(W0_SCALE)
    # [47 kp, 128 p, 2 j, 2048 o]: feature f = (2*kp + j)*128 + p
    w0p = wT0.reshape(47, 2, 128, 2048).transpose(0, 2, 1, 3)
    w_prepped.append(np.ascontiguousarray(w0p.astype(FP8_NP)))
    for k in range(1, 6):
        out_f, in_f = LAYER_SIZES[k]
        wTk = np.zeros((IN_PAD[k], OUT_PAD[k]), np.float32)
        wTk[:in_f, :out_f] = np.asarray(ws[k]).T
        wkp = wTk.reshape(IC[k], 128, OUT_PAD[k]).transpose(1, 0, 2)  # [128p, IC, OUT]
        w_prepped.append(np.ascontiguousarray(wkp.astype(BF16_NP)))

    _Bs, _D, H = _mask_tables()
    hmat = np.ascontiguousarray(H.astype(BF16_NP))  # [4, 32]
    A = np.abs(H).max(1)  # per-channel max_t |H|
    amat = np.zeros((B_CORE * NMASK, B_CORE), np.float32)
    for b in range(B_CORE):
        for i in range(NMASK):
            amat[b * NMASK + i, b] = A[i]
    amat = np.ascontiguousarray(amat.astype(BF16_NP))
    return xs_cores, w_prepped, hmat, amat


_NC_CACHE = {}


def kernel(images, w0, w1, w2, w3, w4, w5):
    global LAST_EXEC_TIME_NS
    ws = [w0, w1, w2, w3, w4, w5]
    xs_cores, w_prepped, hmat, amat = prep_inputs(images, ws)

    trace = os.environ.get("KERNEL_TRACE", "0") == "1"
    if trace:
        _install_ntff_hook()

    if "nc" not in _NC_CACHE:
        _NC_CACHE["nc"] = build_nc()
    nc = _NC_CACHE["nc"]

    in_maps = []
    for c in range(N_CORES):
        m = {"xs": xs_cores[c], "w0t": w_prepped[0], "hconst": hmat, "aconst": amat}
        for k in range(1, 6):
            m[f"w{k}t"] = w_prepped[k]
        in_maps.append(m)

    res = run_bass_kernel_spmd(
        nc, in_maps, core_ids=list(range(N_CORES)), trace=trace
    )
    LAST_EXEC_TIME_NS = res.exec_time_ns
    _NC_CACHE["res"] = res

    # out[c] is [16 feats, 16 batch]; valid feats :10; logits = max_t(V5)/10
    logits = np.concatenate(
        [np.asarray(res.results[c]["out"])[:10, :].T for c in range(N_CORES)], axis=0
    ).astype(np.float32) / np.float32(10.0)
    mx = logits.max(axis=1, keepdims=True)
    sh = logits - mx
    out = sh - np.log(np.exp(sh).sum(axis=1, keepdims=True))
    return out.astype(np.float32)


# revision 10
# speedup vs baseline: 2.5842x; 1.4510x over previous
"""Trainium2 Bass kernel for nn_CaptchaRecognizer (norse-style SNN).

Strategy (pure data-parallel over batch, 8 NeuronCores, 16 images each):

The encoder resets to exactly 0 on spike, so the encoder+LIF0 cascade is a
piecewise-constant function of x alone: only 4 fp32-exact breakpoints B_n
matter, and the LIF0 spike train is EXACTLY LINEAR in the 4 nested threshold
masks u_n = (x >= B_n):   z0[t] = sum_n D[n,t] * u_n   (D host-precomputed).

Hence the layer-0 LI membrane is   V0[t,b,o] = sum_n H[n,t] * Y_n[b,o]   with
Y_n = u_n @ w0^T and H = LI-filtered D.  The 32-timestep spike matmul of the
reference collapses to a 4-channel mask matmul: out rows (b,i) = 16*4 = 64
instead of t*b = 512 — 8x fewer MACs, one stream of w0 from HBM (fp8 x64,
DoubleRow, K-accumulated in PSUM; stationary = masks, moving = w0 columns).

  early exit: the reset-free LIF1 membrane is a triple first-order filter of
            V0 with kernel l1-norm <= 50. A cheap certified bound
            max|V0| <= sum_n (max_t|H_n|) |Y_n|  (PE reduction over the 4
            channels) gives 50*bound < 95 (< threshold 100) => layer 1 never
            spikes => layers 2..5 exactly zero => output the zero logit tile.
  slow path: runtime If; V0 materialized exactly from Y by a tiny PE
            expansion against H, then the original per-layer pipeline
            (LIF via scalar_tensor_tensor steps, LI via tensor_tensor_scan
            linear recurrences, bf16 matmuls for w1..w5).
  output:   max over t of V5/10, log_softmax on host (tiny [128,10]).

Internal dtypes: fp8 masks/w0 (x64), bf16 states/Y/weights, fp32 PSUM.
"""

import os
import sys
import numpy as np
import ml_dtypes

import concourse.bass as bass
import concourse.tile as tile
from concourse import bacc, mybir
from concourse.bass_utils import run_bass_kernel_spmd

AL = mybir.AluOpType
F32 = mybir.dt.float32
BF16 = mybir.dt.bfloat16
FP8 = mybir.dt.float8e4
FP8_NP = mybir.dt.np(mybir.dt.float8e4)
BF16_NP = ml_dtypes.bfloat16
W0_SCALE = 64.0

N_CORES = 8
B_CORE = 16
T = 32
NMASK = 4

LAYER_SIZES = [(2000, 12000), (1500, 2000), (1000, 1500), (500, 1000), (100, 500), (10, 100)]
IN_PAD = [12032, 2048, 1536, 1024, 512, 128]
OUT_PAD = [2048, 1536, 1024, 512, 128, 16]
IC = [94, 16, 12, 8, 4, 1]      # input chunks of 128 (contraction)
MC = [16, 12, 8, 4, 1, 1]       # output chunks (M tiles)
M_SIZE = [128, 128, 128, 128, 128, 16]
KP0 = 47                         # layer-0 DoubleRow k-pairs

LAST_EXEC_TIME_NS = None

DT_DECAY_V = np.float32(0.1)   # DT*TAU_MEM_INV
V_TH = np.float32(1.0)


def _enc_first_spike_step(x_scalar):
    """fp32 encoder sim (exactly mirrors reference arithmetic); first spike step or None."""
    f32 = np.float32
    v = f32(0.0)
    x = f32(x_scalar)
    for t in range(T):
        v = f32(v + f32(DT_DECAY_V * f32(-v + x)))
        if f32(v - V_TH) > 0:
            return t
    return None


def _stage0_tables():
    """Host-precomputed structure of the encoder+LIF0 cascade.

    The encoder resets to exactly 0 on spike, so its spike train is periodic
    with period p(x) = 1 + first_spike_step(x); LIF0's response to a period-p
    train is a fixed pattern G[t, p].  The map x -> LIF0-spike-train is
    piecewise constant in x; we compress it to the breakpoints where the
    pattern actually changes and pack patterns as integer codes.
    Returns (breaks [(B_n, delta_n)...], bit_ts [t for each bit, ascending]).
    """
    f32 = np.float32
    # G[t, c]: c = 0 -> silent input; c = p -> period p
    G = np.zeros((T, 34), np.int64)
    for c in range(1, 33):
        v = f32(0.0)
        i = f32(0.0)
        for t in range(T):
            inp = f32(1.0) if (t + 1) % c == 0 else f32(0.0)
            v_dec = f32(v + f32(DT_DECAY_V * f32(-v + i)))
            i_dec = f32(i * f32(0.8))
            z = 1 if f32(v_dec - V_TH) > 0 else 0
            v = f32(0.0) if z else v_dec
            i = f32(i_dec + inp)
            G[t, c] = z
    bit_ts = [t for t in range(T) if G[t].any()]
    code = {c: sum(int(G[ts, c]) << j for j, ts in enumerate(bit_ts)) for c in range(34)}
    code[33] = 0  # period > 32 == silent
    used = [n for n in range(1, 33) if code[n] != code[n + 1]]

    # fp32-exact breakpoints: B_n = min x with first_spike_step <= n-1
    breaks = []
    for n in used:
        lo = np.float32(1.0).view(np.int32)
        hi = np.float32(20.0).view(np.int32)
        while int(hi) - int(lo) > 1:
            mid = np.int32((int(lo) + int(hi)) // 2)
            s = _enc_first_spike_step(mid.view(np.float32))
            if s is not None and s <= n - 1:
                hi = mid
            else:
                lo = mid
        breaks.append((float(np.int32(hi).view(np.float32)), float(code[n] - code[n + 1])))
    return breaks, bit_ts


def _mask_tables():
    """Per-breakpoint spike-train deltas D [4, T] and LI-filtered H [4, T].

    z0[t] (LIF0 spikes) = sum_n (x >= B_n) * D[n, t]  exactly (nested masks).
    V0[t] (scaled LI0 membrane, V = 10*v) = sum_n H[n, t] * Y_n, with
    H the (i' = 0.8 i' + D; V = 0.9 V + i') double filter of D.
    """
    breaks, bit_ts = _stage0_tables()
    assert len(breaks) == NMASK
    deltas = [d for (_, d) in breaks]
    Bs = [b for (b, _) in breaks]
    # Bs descending: passing B_n implies passing all later (smaller) breakpoints.
    csum = np.cumsum(deltas[::-1])[::-1]  # code when masks n..3 are on

    def bits(c):
        c = int(round(c))
        return np.array([(c >> j) & 1 for j in range(len(bit_ts))], np.float64)

    pats = [bits(c) for c in csum] + [np.zeros(len(bit_ts))]
    D = np.zeros((NMASK, T))
    for n in range(NMASK):
        dv = pats[n] - pats[n + 1]
        for j, t in enumerate(bit_ts):
            D[n, t] = dv[j]
    H = np.zeros((NMASK, T))
    for n in range(NMASK):
        ip = 0.0
        V = 0.0
        for t in range(T):
            ip = 0.8 * ip + D[n, t]
            V = 0.9 * V + ip
            H[n, t] = V
    return Bs, D, H


def _install_ntff_hook():
    import types
    if "antenv.axon_hooks" in sys.modules:
        return
    try:
        mod = types.ModuleType("antenv.axon_hooks")
        mod._hook = None
        mod.set_axon_ntff_profile_hook = lambda h: setattr(mod, "_hook", h)
        mod.get_axon_ntff_profile_hook = lambda: mod._hook
        sys.modules["antenv.axon_hooks"] = mod
        from trn_agent_boot.trn_boot import _ntff_profile_via_ctypes
        mod._hook = _ntff_profile_via_ctypes("/opt/axon/libaxon_pjrt.so")
    except Exception:
        pass


def build_body(tc, ctx, nc, xs_ap, w_aps, h_ap, a_ap, out_ap):
    from contextlib import ExitStack

    Bs, _D, _H = _mask_tables()

    const = ctx.enter_context(tc.tile_pool(name="const", bufs=1))
    psum = ctx.enter_context(tc.tile_pool(name="psum", bufs=8, space="PSUM"))
    ijpool = ctx.enter_context(tc.tile_pool(name="ij", bufs=2))
    spool = ctx.enter_context(tc.tile_pool(name="spikes", bufs=2))

    mask08 = const.tile([128, 512], BF16)
    mask09 = const.tile([128, 512], BF16)

    def emit_masks():
        # decay masks with 0.0 at t=0 of each batch segment (scan segmentation)
        nc.vector.memset(mask08[:], 0.8)
        nc.vector.memset(mask08[:].rearrange("p (b t) -> p b t", b=B_CORE)[:, :, 0:1], 0.0)
        nc.vector.memset(mask09[:], 0.9)
        nc.vector.memset(mask09[:].rearrange("p (b t) -> p b t", b=B_CORE)[:, :, 0:1], 0.0)

    # ---- fast path: masks -> Y = u @ w0^T (64 rows) -> certified LIF1 bound ----
    xr_sb = const.tile([128, KP0, 2, B_CORE], F32)
    nc.sync.dma_start(xr_sb[:], xs_ap)

    # u layout [p, kp, j, (b,i)] b-major: stationary operand of the DR matmul
    u = const.tile([128, KP0, 2, B_CORE, NMASK], FP8)
    for i, bn in enumerate(Bs):
        nc.vector.tensor_scalar(u[:, :, :, :, i], xr_sb[:], float(bn), None, AL.is_ge)

    hc = const.tile([NMASK, T], BF16)      # H for the Else expansion
    nc.sync.dma_start(hc[:], h_ap)
    ac = const.tile([B_CORE * NMASK, B_CORE], BF16)  # bound-reduction matrix
    nc.sync.dma_start(ac[:], a_ap)

    Ysb = const.tile([B_CORE * NMASK, 4, 512], BF16)    # descaled Y, for Else
    absY = const.tile([B_CORE * NMASK, 4, 512], BF16)   # |Y| for the bound

    W0_GROUPS = [(0, 2), (2, 10), (10, 18), (18, 26), (26, 34), (34, 42), (42, 47)]
    with ExitStack() as phase0:
        w0pool = phase0.enter_context(tc.tile_pool(name="w0s", bufs=2))
        ps = [psum.tile([B_CORE * NMASK, 512], F32, name=f"ps{og}", bufs=1) for og in range(4)]
        for g0, g1 in W0_GROUPS:
            wt = w0pool.tile([128, 8, 2, 2048], FP8, name="wt")
            nc.sync.dma_start(
                wt[:, :g1 - g0, :, :],
                w_aps[0][g0:g1].rearrange("g p j o -> p g j o"),
            )
            for kp in range(g0, g1):
                for og in range(4):
                    nc.tensor.matmul(
                        ps[og][:],
                        u[:, kp, :, :, :],
                        wt[:, kp - g0, :, og * 512:(og + 1) * 512],
                        start=(kp == 0),
                        stop=(kp == KP0 - 1),
                        perf_mode=mybir.MatmulPerfMode.DoubleRow,
                    )
        for og in range(4):
            nc.vector.tensor_scalar(
                Ysb[:, og, :], ps[og][:], 1.0 / W0_SCALE, None, AL.mult
            )
            nc.scalar.activation(
                absY[:, og, :], ps[og][:], mybir.ActivationFunctionType.Abs,
                scale=1.0 / W0_SCALE,
            )
        # R[b, o] = sum_i A_i |Y_i[b, o]| via PE: stationary = A [(b,i), b'], then
        # 50 * max R < 95 certifies "layer 1 never spikes".
        rmx = const.tile([B_CORE, 4], F32)
        for og in range(4):
            psr = psum.tile([B_CORE, 512], F32, bufs=1)
            nc.tensor.matmul(psr[:], ac[:], absY[:, og, :], start=True, stop=True)
            nc.vector.tensor_reduce(rmx[:, og:og + 1], psr[:], mybir.AxisListType.X, AL.max)

    from concourse import bass_isa
    amax = const.tile([128, 1], F32)
    nc.vector.memset(amax[:], 0.0)
    nc.vector.tensor_reduce(amax[0:B_CORE, :], rmx[:], mybir.AxisListType.X, AL.max)
    gmax = const.tile([128, 1], F32)
    nc.gpsimd.partition_all_reduce(gmax[:], amax[:], 128, bass_isa.ReduceOp.max)
    gmax_s = const.tile([1, 1], F32)
    nc.vector.tensor_scalar(gmax_s[:], gmax[0:1, 0:1], 50.0, None, AL.mult)
    gmax_i = const.tile([1, 1], mybir.dt.int32)
    nc.vector.tensor_copy(gmax_i[:], gmax_s[:])
    _, (sval,) = nc.values_load_multi_w_load_instructions(
        gmax_i[0:1, 0:1], skip_runtime_bounds_check=True
    )

    # ---- slow-path helpers (baseline per-layer pipeline) ----
    spikes = None  # current layer's input spike tensor, [128, IC[k], 16, 32] bf16

    def lif_phase(k, V, pk):
        nonlocal spikes
        C = MC[k]
        Vv = V[:].rearrange("p m (b t) -> p m b t", t=T)
        S = spool.tile([128, C, B_CORE, T], BF16, tag="S")
        P = pk.tile([128, C, B_CORE], BF16, tag="P")
        Q = pk.tile([128, C, B_CORE], BF16, tag="Q")
        nc.vector.memset(P[:], 0.0)
        nc.vector.memset(Q[:], 0.0)
        for t in range(T):
            nc.vector.scalar_tensor_tensor(P[:], P[:], 0.9, Q[:], AL.mult, AL.add)
            nc.vector.tensor_scalar(S[:, :, :, t], P[:], 100.0, None, AL.is_gt)
            nc.vector.scalar_tensor_tensor(P[:], P[:], 100.0, P[:], AL.is_le, AL.mult)
            nc.vector.scalar_tensor_tensor(Q[:], Q[:], 0.8, Vv[:, :, :, t], AL.mult, AL.add)
        spikes = S

    def layer_phase(k):
        nonlocal spikes
        M = M_SIZE[k]
        with ExitStack() as ph:
            pk = ph.enter_context(tc.tile_pool(name=f"phase{k + 1}", bufs=1))
            if k == 5:
                V = pk.tile([M, 512], F32, tag="V5")
            else:
                V = pk.tile([128, MC[k], 512], BF16, tag=f"V{k}")

            wk_sb = pk.tile([128, IC[k], OUT_PAD[k]], BF16, tag=f"w{k}")
            nc.sync.dma_start(wk_sb[:], w_aps[k])

            for m in range(MC[k]):
                ps = psum.tile([128, 512], F32, bufs=2)
                for kc in range(IC[k]):
                    nc.tensor.matmul(
                        ps[:M, :],
                        wk_sb[:, kc, m * 128:m * 128 + M],
                        spikes[:, kc, :, :],
                        start=(kc == 0),
                        stop=(kc == IC[k] - 1),
                    )
                j_src = ps[:M, :]
                ij = ijpool.tile([128, 512], BF16)
                nc.vector.tensor_tensor_scan(ij[:M, :], mask08[:M, :], j_src, 0.0, AL.mult, AL.add)
                if k == 5:
                    nc.vector.tensor_tensor_scan(V[:, :], mask09[:M, :], ij[:M, :], 0.0, AL.mult, AL.add)
                else:
                    nc.vector.tensor_tensor_scan(V[:, m, :], mask09[:, :], ij[:, :], 0.0, AL.mult, AL.add)

            if k == 5:
                rmax = pk.tile([M, B_CORE], F32)
                nc.vector.tensor_reduce(
                    rmax[:], V[:].rearrange("p (b t) -> p b t", b=B_CORE),
                    mybir.AxisListType.X, AL.max,
                )
                nc.sync.dma_start(out_ap, rmax[:])
            else:
                lif_phase(k, V, pk)

    with tc.If(sval < 95) as cmp:
        zero_out = const.tile([M_SIZE[5], B_CORE], F32)
        nc.vector.memset(zero_out[:], 0.0)
        nc.sync.dma_start(out_ap, zero_out[:])
    with cmp.Else():
        emit_masks()
        with ExitStack() as phl:
            pl = phl.enter_context(tc.tile_pool(name="lif1", bufs=1))
            # transpose Y to partitions = i for PE expansion against H
            Yt = pl.tile([NMASK, B_CORE, 4, 512], BF16, tag="Yt")
            for b in range(B_CORE):
                nc.sync.dma_start(
                    Yt[:, b, :, :], Ysb[b * NMASK:(b + 1) * NMASK, :, :]
                )
            V0 = pl.tile([128, MC[0], 512], BF16, tag="V0")
            for m in range(MC[0]):
                psv = psum.tile([128, 512], F32, bufs=1)
                for b in range(B_CORE):
                    nc.tensor.matmul(
                        psv[:, b * T:(b + 1) * T],
                        Yt[:, b, m // 4, (m % 4) * 128:(m % 4) * 128 + 128],
                        hc[:],
                        start=True, stop=True,
                    )
                nc.scalar.activation(
                    V0[:, m, :], psv[:], mybir.ActivationFunctionType.Copy, scale=1.0
                )
            lif_phase(0, V0, pl)
        for k in range(1, 6):
            layer_phase(k)


def build_nc():
    from contextlib import ExitStack

    nc = bacc.Bacc("TRN2", debug=False, num_devices=N_CORES)
    xs = nc.dram_tensor("xs", [128, KP0, 2, B_CORE], F32, kind="ExternalInput")
    w_t = [nc.dram_tensor("w0t", [KP0, 128, 2, 2048], FP8, kind="ExternalInput")]
    for k in range(1, 6):
        w_t.append(
            nc.dram_tensor(f"w{k}t", [128, IC[k], OUT_PAD[k]], BF16, kind="ExternalInput")
        )
    hconst = nc.dram_tensor("hconst", [NMASK, T], BF16, kind="ExternalInput")
    aconst = nc.dram_tensor("aconst", [B_CORE * NMASK, B_CORE], BF16, kind="ExternalInput")
    out = nc.dram_tensor("out", [M_SIZE[5], B_CORE], F32, kind="ExternalOutput")

    with tile.TileContext(nc) as tc, ExitStack() as ctx:
        build_body(tc, ctx, nc, xs.ap(), [w.ap() for w in w_t],
                   hconst.ap(), aconst.ap(), out.ap())
    nc.compile()
    return nc


def prep_inputs(images, ws):
    """Host-side marshalling: pad/transpose/cast weights, rearrange images."""
    x = np.asarray(images).reshape(128, -1).astype(np.float32)  # [B, 12000]
    xs = np.zeros((128, 12032), np.float32)
    xs[:, :12000] = x
    # [p, kp, j, b] with feature f = (2*kp + j)*128 + p
    xs_r = xs.reshape(128, 47, 2, 128).transpose(3, 1, 2, 0)  # [128p, 47, 2, 128b]
    xs_cores = [
        np.ascontiguousarray(xs_r[:, :, :, c * B_CORE:(c + 1) * B_CORE])
        for c in range(N_CORES)
    ]

    w_prepped = []
    wT0 = np.zeros((12032, 2048), np.float32)
    wT0[:12000, :2000] = np.asarray(ws[0]).T * np.float32(W0_SCALE)
    # [47 kp, 128 p, 2 j, 2048 o]: feature f = (2*kp + j)*128 + p
    w0p = wT0.reshape(47, 2, 128, 2048).transpose(0, 2, 1, 3)
    w_prepped.append(np.ascontiguousarray(w0p.astype(FP8_NP)))
    for k in range(1, 6):
        out_f, in_f = LAYER_SIZES[k]
        wTk = np.zeros((IN_PAD[k], OUT_PAD[k]), np.float32)
        wTk[:in_f, :out_f] = np.asarray(ws[k]).T
        wkp = wTk.reshape(IC[k], 128, OUT_PAD[k]).transpose(1, 0, 2)  # [128p, IC, OUT]
        w_prepped.append(np.ascontiguousarray(wkp.astype(BF16_NP)))

    _Bs, _D, H = _mask_tables()
    hmat = np.ascontiguousarray(H.astype(BF16_NP))  # [4, 32]
    A = np.abs(H).max(1)  # per-channel max_t |H|
    amat = np.zeros((B_CORE * NMASK, B_CORE), np.float32)
    for b in range(B_CORE):
        for i in range(NMASK):
            amat[b * NMASK + i, b] = A[i]
    amat = np.ascontiguousarray(amat.astype(BF16_NP))
    return xs_cores, w_prepped, hmat, amat


_NC_CACHE = {}


def kernel(images, w0, w1, w2, w3, w4, w5):
    global LAST_EXEC_TIME_NS
    ws = [w0, w1, w2, w3, w4, w5]
    xs_cores, w_prepped, hmat, amat = prep_inputs(images, ws)

    trace = os.environ.get("KERNEL_TRACE", "0") == "1"
    if trace:
        _install_ntff_hook()

    if "nc" not in _NC_CACHE:
        _NC_CACHE["nc"] = build_nc()
    nc = _NC_CACHE["nc"]

    in_maps = []
    for c in range(N_CORES):
        m = {"xs": xs_cores[c], "w0t": w_prepped[0], "hconst": hmat, "aconst": amat}
        for k in range(1, 6):
            m[f"w{k}t"] = w_prepped[k]
        in_maps.append(m)

    res = run_bass_kernel_spmd(
        nc, in_maps, core_ids=list(range(N_CORES)), trace=trace
    )
    LAST_EXEC_TIME_NS = res.exec_time_ns
    _NC_CACHE["res"] = res

    # out[c] is [16 feats, 16 batch]; valid feats :10; logits = max_t(V5)/10
    logits = np.concatenate(
        [np.asarray(res.results[c]["out"])[:10, :].T for c in range(N_CORES)], axis=0
    ).astype(np.float32) / np.float32(10.0)
    mx = logits.max(axis=1, keepdims=True)
    sh = logits - mx
    out = sh - np.log(np.exp(sh).sum(axis=1, keepdims=True))
    return out.astype(np.float32)


# revision 13
# speedup vs baseline: 3.1868x; 1.2332x over previous
"""Trainium2 Bass kernel for nn_CaptchaRecognizer (norse-style SNN).

Strategy (pure data-parallel over batch, 8 NeuronCores, 16 images each):

The encoder resets to exactly 0 on spike, so the encoder+LIF0 cascade is a
piecewise-constant function of x alone: only 4 fp32-exact breakpoints B_n
matter, and the LIF0 spike train is EXACTLY LINEAR in the 4 nested threshold
masks u_n = (x >= B_n):   z0[t] = sum_n D[n,t] * u_n   (D host-precomputed).

Hence the layer-0 LI membrane is   V0[t,b,o] = sum_n H[n,t] * Y_n[b,o]   with
Y_n = u_n @ w0^T and H = LI-filtered D.  The 32-timestep spike matmul of the
reference collapses to a 4-channel mask matmul: out rows (b,i) = 16*4 = 64
instead of t*b = 512 — 8x fewer MACs, one stream of w0 from HBM (fp8 x64,
DoubleRow, K-accumulated in PSUM; stationary = masks, moving = w0 columns).

  early exit: the reset-free LIF1 membrane is a triple first-order filter of
            V0 with kernel l1-norm <= 50. A cheap certified bound
            max|V0| <= sum_n (max_t|H_n|) |Y_n|  (PE reduction over the 4
            channels) gives 50*bound < 95 (< threshold 100) => layer 1 never
            spikes => layers 2..5 exactly zero => output the zero logit tile.
  slow path: runtime If; V0 materialized exactly from Y by a tiny PE
            expansion against H, then the original per-layer pipeline
            (LIF via scalar_tensor_tensor steps, LI via tensor_tensor_scan
            linear recurrences, bf16 matmuls for w1..w5).
  output:   max over t of V5/10, log_softmax on host (tiny [128,10]).

Internal dtypes: fp8 masks/w0 (x64), bf16 states/Y/weights, fp32 PSUM.
"""

import os
import sys
import numpy as np
import ml_dtypes

import concourse.bass as bass
import concourse.tile as tile
from concourse import bacc, mybir
from concourse.bass_utils import run_bass_kernel_spmd

AL = mybir.AluOpType
F32 = mybir.dt.float32
BF16 = mybir.dt.bfloat16
FP8 = mybir.dt.float8e4
FP8_NP = mybir.dt.np(mybir.dt.float8e4)
BF16_NP = ml_dtypes.bfloat16
W0_SCALE = 64.0

N_CORES = 8
B_CORE = 16
T = 32
NMASK = 4

LAYER_SIZES = [(2000, 12000), (1500, 2000), (1000, 1500), (500, 1000), (100, 500), (10, 100)]
IN_PAD = [12032, 2048, 1536, 1024, 512, 128]
OUT_PAD = [2048, 1536, 1024, 512, 128, 16]
IC = [94, 16, 12, 8, 4, 1]      # input chunks of 128 (contraction)
MC = [16, 12, 8, 4, 1, 1]       # output chunks (M tiles)
M_SIZE = [128, 128, 128, 128, 128, 16]
KP0 = 47                         # layer-0 DoubleRow k-pairs
KSLOT = 12                       # gather slots per partition (max actives)
NFROW = 12160                    # padded feature rows for the gather tables

LAST_EXEC_TIME_NS = None

DT_DECAY_V = np.float32(0.1)   # DT*TAU_MEM_INV
V_TH = np.float32(1.0)


def _enc_first_spike_step(x_scalar):
    """fp32 encoder sim (exactly mirrors reference arithmetic); first spike step or None."""
    f32 = np.float32
    v = f32(0.0)
    x = f32(x_scalar)
    for t in range(T):
        v = f32(v + f32(DT_DECAY_V * f32(-v + x)))
        if f32(v - V_TH) > 0:
            return t
    return None


def _stage0_tables():
    """Host-precomputed structure of the encoder+LIF0 cascade.

    The encoder resets to exactly 0 on spike, so its spike train is periodic
    with period p(x) = 1 + first_spike_step(x); LIF0's response to a period-p
    train is a fixed pattern G[t, p].  The map x -> LIF0-spike-train is
    piecewise constant in x; we compress it to the breakpoints where the
    pattern actually changes and pack patterns as integer codes.
    Returns (breaks [(B_n, delta_n)...], bit_ts [t for each bit, ascending]).
    """
    f32 = np.float32
    # G[t, c]: c = 0 -> silent input; c = p -> period p
    G = np.zeros((T, 34), np.int64)
    for c in range(1, 33):
        v = f32(0.0)
        i = f32(0.0)
        for t in range(T):
            inp = f32(1.0) if (t + 1) % c == 0 else f32(0.0)
            v_dec = f32(v + f32(DT_DECAY_V * f32(-v + i)))
            i_dec = f32(i * f32(0.8))
            z = 1 if f32(v_dec - V_TH) > 0 else 0
            v = f32(0.0) if z else v_dec
            i = f32(i_dec + inp)
            G[t, c] = z
    bit_ts = [t for t in range(T) if G[t].any()]
    code = {c: sum(int(G[ts, c]) << j for j, ts in enumerate(bit_ts)) for c in range(34)}
    code[33] = 0  # period > 32 == silent
    used = [n for n in range(1, 33) if code[n] != code[n + 1]]

    # fp32-exact breakpoints: B_n = min x with first_spike_step <= n-1
    breaks = []
    for n in used:
        lo = np.float32(1.0).view(np.int32)
        hi = np.float32(20.0).view(np.int32)
        while int(hi) - int(lo) > 1:
            mid = np.int32((int(lo) + int(hi)) // 2)
            s = _enc_first_spike_step(mid.view(np.float32))
            if s is not None and s <= n - 1:
                hi = mid
            else:
                lo = mid
        breaks.append((float(np.int32(hi).view(np.float32)), float(code[n] - code[n + 1])))
    return breaks, bit_ts


def _mask_tables():
    """Per-breakpoint spike-train deltas D [4, T] and LI-filtered H [4, T].

    z0[t] (LIF0 spikes) = sum_n (x >= B_n) * D[n, t]  exactly (nested masks).
    V0[t] (scaled LI0 membrane, V = 10*v) = sum_n H[n, t] * Y_n, with
    H the (i' = 0.8 i' + D; V = 0.9 V + i') double filter of D.
    """
    breaks, bit_ts = _stage0_tables()
    assert len(breaks) == NMASK
    deltas = [d for (_, d) in breaks]
    Bs = [b for (b, _) in breaks]
    # Bs descending: passing B_n implies passing all later (smaller) breakpoints.
    csum = np.cumsum(deltas[::-1])[::-1]  # code when masks n..3 are on

    def bits(c):
        c = int(round(c))
        return np.array([(c >> j) & 1 for j in range(len(bit_ts))], np.float64)

    pats = [bits(c) for c in csum] + [np.zeros(len(bit_ts))]
    D = np.zeros((NMASK, T))
    for n in range(NMASK):
        dv = pats[n] - pats[n + 1]
        for j, t in enumerate(bit_ts):
            D[n, t] = dv[j]
    H = np.zeros((NMASK, T))
    for n in range(NMASK):
        ip = 0.0
        V = 0.0
        for t in range(T):
            ip = 0.8 * ip + D[n, t]
            V = 0.9 * V + ip
            H[n, t] = V
    return Bs, D, H


def _install_ntff_hook():
    import types
    if "antenv.axon_hooks" in sys.modules:
        return
    try:
        mod = types.ModuleType("antenv.axon_hooks")
        mod._hook = None
        mod.set_axon_ntff_profile_hook = lambda h: setattr(mod, "_hook", h)
        mod.get_axon_ntff_profile_hook = lambda: mod._hook
        sys.modules["antenv.axon_hooks"] = mod
        from trn_agent_boot.trn_boot import _ntff_profile_via_ctypes
        mod._hook = _ntff_profile_via_ctypes("/opt/axon/libaxon_pjrt.so")
    except Exception:
        pass


def build_body(tc, ctx, nc, xs_ap, w_aps, h_ap, a_ap, out_ap, cv_ap, pio_ap, xg_ap, w0g_ap):
    from contextlib import ExitStack

    Bs, _D, _H = _mask_tables()

    const = ctx.enter_context(tc.tile_pool(name="const", bufs=1))
    psum = ctx.enter_context(tc.tile_pool(name="psum", bufs=8, space="PSUM"))
    ijpool = ctx.enter_context(tc.tile_pool(name="ij", bufs=2))
    spool = ctx.enter_context(tc.tile_pool(name="spikes", bufs=2))

    mask08 = const.tile([128, 512], BF16)
    mask09 = const.tile([128, 512], BF16)

    def emit_masks():
        # decay masks with 0.0 at t=0 of each batch segment (scan segmentation)
        nc.vector.memset(mask08[:], 0.8)
        nc.vector.memset(mask08[:].rearrange("p (b t) -> p b t", b=B_CORE)[:, :, 0:1], 0.0)
        nc.vector.memset(mask09[:], 0.9)
        nc.vector.memset(mask09[:].rearrange("p (b t) -> p b t", b=B_CORE)[:, :, 0:1], 0.0)

    # ---- fast path: per-partition compaction of active features ----
    # A feature (p, c) is active iff any of its 16 images crosses the lowest
    # breakpoint. Per partition there are <= KSLOT active chunks (overflow ->
    # certified fallback to the dense slow path); gather only those w0 rows.
    xr_sb = const.tile([128, KP0, 2, B_CORE], F32)
    nc.sync.dma_start(xr_sb[:], xs_ap)
    cv = const.tile([128, KP0 * 2], F32)       # c+1 per (kp, j), const
    nc.sync.dma_start(cv[:], cv_ap)
    pio = const.tile([128, B_CORE], F32)       # partition index, const
    nc.sync.dma_start(pio[:], pio_ap)
    hc = const.tile([NMASK, T], BF16)      # H for the Else expansion
    nc.sync.dma_start(hc[:], h_ap)
    ac = const.tile([B_CORE * NMASK, B_CORE], BF16)  # bound-reduction matrix
    nc.sync.dma_start(ac[:], a_ap)

    Ysb = const.tile([B_CORE * NMASK, 4, 512], BF16)    # descaled Y (Else only)
    absY = const.tile([B_CORE * NMASK, 4, 512], BF16)   # |Y| for the bound

    xm = const.tile([128, KP0 * 2], F32)
    nc.vector.tensor_reduce(
        xm[:].rearrange("p (a b) -> p a b", b=2), xr_sb[:], mybir.AxisListType.X, AL.max
    )
    act = const.tile([128, KP0 * 2], F32)
    nc.vector.tensor_scalar(act[:], xm[:], float(Bs[NMASK - 1]), None, AL.is_ge)
    ones94 = const.tile([128, KP0 * 2], F32)
    nc.vector.memset(ones94[:], 1.0)
    incl = const.tile([128, KP0 * 2], F32)
    nc.vector.tensor_tensor_scan(incl[:], ones94[:], act[:], 0.0, AL.mult, AL.add)
    excl = const.tile([128, KP0 * 2], F32)
    nc.vector.tensor_tensor(excl[:], incl[:], act[:], AL.subtract)
    acv = const.tile([128, KP0 * 2], F32)   # act * (c+1)
    nc.vector.tensor_tensor(acv[:], act[:], cv[:], AL.mult)

    # ck[p, k] = chunk index of k-th active chunk of partition p, or 94 (pad)
    ck = const.tile([128, KSLOT], F32)
    tmpa = const.tile([128, KP0 * 2], F32)
    for k in range(KSLOT):
        nc.vector.scalar_tensor_tensor(
            tmpa[:], excl[:], float(k), acv[:], AL.is_equal, AL.mult
        )
        nc.vector.tensor_reduce(ck[:, k:k + 1], tmpa[:], mybir.AxisListType.X, AL.add)
    tmpk = const.tile([128, KSLOT], F32)
    nc.vector.tensor_scalar(tmpk[:], ck[:], 0.0, 95.0, AL.is_equal, AL.mult)
    nc.vector.tensor_tensor(ck[:], ck[:], tmpk[:], AL.add)
    nc.vector.tensor_scalar(ck[:], ck[:], 1.0, None, AL.subtract)

    fof = const.tile([128, KSLOT], F32)     # row index = c*128 + p
    nc.vector.scalar_tensor_tensor(fof[:], ck[:], 128.0, pio[:, :KSLOT], AL.mult, AL.add)
    foi = const.tile([128, KSLOT], mybir.dt.int32)
    nc.vector.tensor_copy(foi[:], fof[:])

    xg = const.tile([128, KSLOT, B_CORE], F32)
    wg = const.tile([128, KSLOT, 2048], FP8)
    for k in range(KSLOT):
        nc.gpsimd.indirect_dma_start(
            out=xg[:, k, :], out_offset=None, in_=xg_ap,
            in_offset=bass.IndirectOffsetOnAxis(ap=foi[:, k:k + 1], axis=0),
        )
        nc.gpsimd.indirect_dma_start(
            out=wg[:, k, :], out_offset=None, in_=w0g_ap,
            in_offset=bass.IndirectOffsetOnAxis(ap=foi[:, k:k + 1], axis=0),
        )

    uc = const.tile([128, KSLOT // 2, 2, B_CORE, NMASK], FP8)
    xgv = xg[:].rearrange("p (t j) b -> p t j b", j=2)
    for i, bn in enumerate(Bs):
        nc.vector.tensor_scalar(uc[:, :, :, :, i], xgv, float(bn), None, AL.is_ge)

    wgv = wg[:].rearrange("p (t j) o -> p t j o", j=2)
    ps = [psum.tile([B_CORE * NMASK, 512], F32, name=f"ps{og}", bufs=1) for og in range(4)]
    for t in range(KSLOT // 2):
        for og in range(4):
            nc.tensor.matmul(
                ps[og][:],
                uc[:, t, :, :, :],
                wgv[:, t, :, og * 512:(og + 1) * 512],
                start=(t == 0),
                stop=(t == KSLOT // 2 - 1),
                perf_mode=mybir.MatmulPerfMode.DoubleRow,
            )
    for og in range(4):
        nc.scalar.activation(
            absY[:, og, :], ps[og][:], mybir.ActivationFunctionType.Abs,
            scale=1.0 / W0_SCALE,
        )
    # R[b, o] = sum_i A_i |Y_i[b, o]| via PE: stationary = A [(b,i), b'], then
    # 50 * max R < 95 certifies "layer 1 never spikes".
    rmx = const.tile([B_CORE, 4], F32)
    for og in range(4):
        psr = psum.tile([B_CORE, 512], F32, bufs=1)
        nc.tensor.matmul(psr[:], ac[:], absY[:, og, :], start=True, stop=True)
        nc.vector.tensor_reduce(rmx[:, og:og + 1], psr[:], mybir.AxisListType.X, AL.max)

    from concourse import bass_isa
    amax = const.tile([128, 1], F32)
    nc.vector.memset(amax[:], 0.0)
    nc.vector.tensor_reduce(amax[0:B_CORE, :], rmx[:], mybir.AxisListType.X, AL.max)
    gmax = const.tile([128, 1], F32)
    nc.gpsimd.partition_all_reduce(gmax[:], amax[:], 128, bass_isa.ReduceOp.max)
    # slot overflow (max_p cnt > KSLOT) forces the dense slow path: +1000
    cmax = const.tile([128, 1], F32)
    nc.gpsimd.partition_all_reduce(cmax[:], incl[:, KP0 * 2 - 1:], 128, bass_isa.ReduceOp.max)
    ovf = const.tile([1, 1], F32)
    nc.vector.tensor_scalar(ovf[:], cmax[0:1, :], float(KSLOT), 1000.0, AL.is_gt, AL.mult)
    gmax_s = const.tile([1, 1], F32)
    nc.vector.scalar_tensor_tensor(gmax_s[:], gmax[0:1, 0:1], 50.0, ovf[:], AL.mult, AL.add)
    gmax_i = const.tile([1, 1], mybir.dt.int32)
    nc.vector.tensor_copy(gmax_i[:], gmax_s[:])
    _, (sval,) = nc.values_load_multi_w_load_instructions(
        gmax_i[0:1, 0:1], skip_runtime_bounds_check=True
    )

    def emit_dense_Y():
        # exact dense recomputation of Y (covers slot overflow), Else only
        with ExitStack() as phd:
            pd = phd.enter_context(tc.tile_pool(name="dense0", bufs=1))
            uf = pd.tile([128, KP0, 2, B_CORE, NMASK], FP8, tag="uf")
            for i, bn in enumerate(Bs):
                nc.vector.tensor_scalar(uf[:, :, :, :, i], xr_sb[:], float(bn), None, AL.is_ge)
            w0pool = phd.enter_context(tc.tile_pool(name="w0s", bufs=2))
            W0_GROUPS = [(0, 2), (2, 10), (10, 18), (18, 26), (26, 34), (34, 42), (42, 47)]
            psd = [psum.tile([B_CORE * NMASK, 512], F32, name=f"ps{og}", bufs=1) for og in range(4)]
            for g0, g1 in W0_GROUPS:
                wt = w0pool.tile([128, 8, 2, 2048], FP8, name="wt")
                nc.sync.dma_start(
                    wt[:, :g1 - g0, :, :],
                    w_aps[0][g0:g1].rearrange("g p j o -> p g j o"),
                )
                for kp in range(g0, g1):
                    for og in range(4):
                        nc.tensor.matmul(
                            psd[og][:],
                            uf[:, kp, :, :, :],
                            wt[:, kp - g0, :, og * 512:(og + 1) * 512],
                            start=(kp == 0),
                            stop=(kp == KP0 - 1),
                            perf_mode=mybir.MatmulPerfMode.DoubleRow,
                        )
            for og in range(4):
                nc.vector.tensor_scalar(
                    Ysb[:, og, :], psd[og][:], 1.0 / W0_SCALE, None, AL.mult
                )

    # ---- slow-path helpers (baseline per-layer pipeline) ----
    spikes = None  # current layer's input spike tensor, [128, IC[k], 16, 32] bf16

    def lif_phase(k, V, pk):
        nonlocal spikes
        C = MC[k]
        Vv = V[:].rearrange("p m (b t) -> p m b t", t=T)
        S = spool.tile([128, C, B_CORE, T], BF16, tag="S")
        P = pk.tile([128, C, B_CORE], BF16, tag="P")
        Q = pk.tile([128, C, B_CORE], BF16, tag="Q")
        nc.vector.memset(P[:], 0.0)
        nc.vector.memset(Q[:], 0.0)
        for t in range(T):
            nc.vector.scalar_tensor_tensor(P[:], P[:], 0.9, Q[:], AL.mult, AL.add)
            nc.vector.tensor_scalar(S[:, :, :, t], P[:], 100.0, None, AL.is_gt)
            nc.vector.scalar_tensor_tensor(P[:], P[:], 100.0, P[:], AL.is_le, AL.mult)
            nc.vector.scalar_tensor_tensor(Q[:], Q[:], 0.8, Vv[:, :, :, t], AL.mult, AL.add)
        spikes = S

    def layer_phase(k):
        nonlocal spikes
        M = M_SIZE[k]
        with ExitStack() as ph:
            pk = ph.enter_context(tc.tile_pool(name=f"phase{k + 1}", bufs=1))
            if k == 5:
                V = pk.tile([M, 512], F32, tag="V5")
            else:
                V = pk.tile([128, MC[k], 512], BF16, tag=f"V{k}")

            wk_sb = pk.tile([128, IC[k], OUT_PAD[k]], BF16, tag=f"w{k}")
            nc.sync.dma_start(wk_sb[:], w_aps[k])

            for m in range(MC[k]):
                ps = psum.tile([128, 512], F32, bufs=2)
                for kc in range(IC[k]):
                    nc.tensor.matmul(
                        ps[:M, :],
                        wk_sb[:, kc, m * 128:m * 128 + M],
                        spikes[:, kc, :, :],
                        start=(kc == 0),
                        stop=(kc == IC[k] - 1),
                    )
                j_src = ps[:M, :]
                ij = ijpool.tile([128, 512], BF16)
                nc.vector.tensor_tensor_scan(ij[:M, :], mask08[:M, :], j_src, 0.0, AL.mult, AL.add)
                if k == 5:
                    nc.vector.tensor_tensor_scan(V[:, :], mask09[:M, :], ij[:M, :], 0.0, AL.mult, AL.add)
                else:
                    nc.vector.tensor_tensor_scan(V[:, m, :], mask09[:, :], ij[:, :], 0.0, AL.mult, AL.add)

            if k == 5:
                rmax = pk.tile([M, B_CORE], F32)
                nc.vector.tensor_reduce(
                    rmax[:], V[:].rearrange("p (b t) -> p b t", b=B_CORE),
                    mybir.AxisListType.X, AL.max,
                )
                nc.sync.dma_start(out_ap, rmax[:])
            else:
                lif_phase(k, V, pk)

    with tc.If(sval < 95) as cmp:
        zero_out = const.tile([M_SIZE[5], B_CORE], F32)
        nc.vector.memset(zero_out[:], 0.0)
        nc.sync.dma_start(out_ap, zero_out[:])
    with cmp.Else():
        emit_masks()
        emit_dense_Y()
        with ExitStack() as phl:
            pl = phl.enter_context(tc.tile_pool(name="lif1", bufs=1))
            # transpose Y to partitions = i for PE expansion against H
            Yt = pl.tile([NMASK, B_CORE, 4, 512], BF16, tag="Yt")
            for b in range(B_CORE):
                nc.sync.dma_start(
                    Yt[:, b, :, :], Ysb[b * NMASK:(b + 1) * NMASK, :, :]
                )
            V0 = pl.tile([128, MC[0], 512], BF16, tag="V0")
            for m in range(MC[0]):
                psv = psum.tile([128, 512], F32, bufs=1)
                for b in range(B_CORE):
                    nc.tensor.matmul(
                        psv[:, b * T:(b + 1) * T],
                        Yt[:, b, m // 4, (m % 4) * 128:(m % 4) * 128 + 128],
                        hc[:],
                        start=True, stop=True,
                    )
                nc.scalar.activation(
                    V0[:, m, :], psv[:], mybir.ActivationFunctionType.Copy, scale=1.0
                )
            lif_phase(0, V0, pl)
        for k in range(1, 6):
            layer_phase(k)


def build_nc():
    from contextlib import ExitStack

    nc = bacc.Bacc("TRN2", debug=False, num_devices=N_CORES)
    xs = nc.dram_tensor("xs", [128, KP0, 2, B_CORE], F32, kind="ExternalInput")
    w_t = [nc.dram_tensor("w0t", [KP0, 128, 2, 2048], FP8, kind="ExternalInput")]
    for k in range(1, 6):
        w_t.append(
            nc.dram_tensor(f"w{k}t", [128, IC[k], OUT_PAD[k]], BF16, kind="ExternalInput")
        )
    hconst = nc.dram_tensor("hconst", [NMASK, T], BF16, kind="ExternalInput")
    aconst = nc.dram_tensor("aconst", [B_CORE * NMASK, B_CORE], BF16, kind="ExternalInput")
    cvconst = nc.dram_tensor("cvconst", [128, KP0 * 2], F32, kind="ExternalInput")
    pioconst = nc.dram_tensor("pioconst", [128, B_CORE], F32, kind="ExternalInput")
    xgath = nc.dram_tensor("xgath", [NFROW, B_CORE], F32, kind="ExternalInput")
    w0gath = nc.dram_tensor("w0gath", [NFROW, 2048], FP8, kind="ExternalInput")
    out = nc.dram_tensor("out", [M_SIZE[5], B_CORE], F32, kind="ExternalOutput")

    with tile.TileContext(nc) as tc, ExitStack() as ctx:
        build_body(tc, ctx, nc, xs.ap(), [w.ap() for w in w_t],
                   hconst.ap(), aconst.ap(), out.ap(),
                   cvconst.ap(), pioconst.ap(), xgath.ap(), w0gath.ap())
    nc.compile()
    return nc


def prep_inputs(images, ws):
    """Host-side marshalling: pad/transpose/cast weights, rearrange images."""
    x = np.asarray(images).reshape(128, -1).astype(np.float32)  # [B, 12000]
    xs = np.zeros((128, 12032), np.float32)
    xs[:, :12000] = x
    # [p, kp, j, b] with feature f = (2*kp + j)*128 + p
    xs_r = xs.reshape(128, 47, 2, 128).transpose(3, 1, 2, 0)  # [128p, 47, 2, 128b]
    xs_cores = [
        np.ascontiguousarray(xs_r[:, :, :, c * B_CORE:(c + 1) * B_CORE])
        for c in range(N_CORES)
    ]

    w_prepped = []
    wT0 = np.zeros((12032, 2048), np.float32)
    wT0[:12000, :2000] = np.asarray(ws[0]).T * np.float32(W0_SCALE)
    # [47 kp, 128 p, 2 j, 2048 o]: feature f = (2*kp + j)*128 + p
    w0p = wT0.reshape(47, 2, 128, 2048).transpose(0, 2, 1, 3)
    w_prepped.append(np.ascontiguousarray(w0p.astype(FP8_NP)))
    for k in range(1, 6):
        out_f, in_f = LAYER_SIZES[k]
        wTk = np.zeros((IN_PAD[k], OUT_PAD[k]), np.float32)
        wTk[:in_f, :out_f] = np.asarray(ws[k]).T
        wkp = wTk.reshape(IC[k], 128, OUT_PAD[k]).transpose(1, 0, 2)  # [128p, IC, OUT]
        w_prepped.append(np.ascontiguousarray(wkp.astype(BF16_NP)))

    xgath_cores = []
    for c in range(N_CORES):
        xgc = np.zeros((NFROW, B_CORE), np.float32)
        xgc[:12032] = xs[c * B_CORE:(c + 1) * B_CORE].T
        xgath_cores.append(np.ascontiguousarray(xgc))
    w0gath = np.zeros((NFROW, 2048), np.float32)
    w0gath[:12032] = wT0
    w0gath = np.ascontiguousarray(w0gath.astype(FP8_NP))
    cvmat = np.ascontiguousarray(
        np.tile(np.arange(1, KP0 * 2 + 1, dtype=np.float32)[None, :], (128, 1))
    )
    piomat = np.ascontiguousarray(
        np.tile(np.arange(128, dtype=np.float32)[:, None], (1, B_CORE))
    )
    _Bs, _D, H = _mask_tables()
    hmat = np.ascontiguousarray(H.astype(BF16_NP))  # [4, 32]
    A = np.abs(H).max(1)  # per-channel max_t |H|
    amat = np.zeros((B_CORE * NMASK, B_CORE), np.float32)
    for b in range(B_CORE):
        for i in range(NMASK):
            amat[b * NMASK + i, b] = A[i]
    amat = np.ascontiguousarray(amat.astype(BF16_NP))
    return xs_cores, w_prepped, hmat, amat, cvmat, piomat, xgath_cores, w0gath


_NC_CACHE = {}


def kernel(images, w0, w1, w2, w3, w4, w5):
    global LAST_EXEC_TIME_NS
    ws = [w0, w1, w2, w3, w4, w5]
    (xs_cores, w_prepped, hmat, amat, cvmat, piomat,
     xgath_cores, w0gath) = prep_inputs(images, ws)

    trace = os.environ.get("KERNEL_TRACE", "0") == "1"
    if trace:
        _install_ntff_hook()

    if "nc" not in _NC_CACHE:
        _NC_CACHE["nc"] = build_nc()
    nc = _NC_CACHE["nc"]

    in_maps = []
    for c in range(N_CORES):
        m = {"xs": xs_cores[c], "w0t": w_prepped[0], "hconst": hmat, "aconst": amat,
             "cvconst": cvmat, "pioconst": piomat, "xgath": xgath_cores[c],
             "w0gath": w0gath}
        for k in range(1, 6):
            m[f"w{k}t"] = w_prepped[k]
        in_maps.append(m)

    res = run_bass_kernel_spmd(
        nc, in_maps, core_ids=list(range(N_CORES)), trace=trace
    )
    LAST_EXEC_TIME_NS = res.exec_time_ns
    _NC_CACHE["res"] = res

    # out[c] is [16 feats, 16 batch]; valid feats :10; logits = max_t(V5)/10
    logits = np.concatenate(
        [np.asarray(res.results[c]["out"])[:10, :].T for c in range(N_CORES)], axis=0
    ).astype(np.float32) / np.float32(10.0)
    mx = logits.max(axis=1, keepdims=True)
    sh = logits - mx
    out = sh - np.log(np.exp(sh).sum(axis=1, keepdims=True))
    return out.astype(np.float32)


# revision 16
# speedup vs baseline: 3.3401x; 1.0481x over previous
"""Trainium2 Bass kernel for nn_CaptchaRecognizer (norse-style SNN).

Strategy (pure data-parallel over batch, 8 NeuronCores, 16 images each):

The encoder resets to exactly 0 on spike, so the encoder+LIF0 cascade is a
piecewise-constant function of x alone: only 4 fp32-exact breakpoints B_n
matter, and the LIF0 spike train is EXACTLY LINEAR in the 4 nested threshold
masks u_n = (x >= B_n):   z0[t] = sum_n D[n,t] * u_n   (D host-precomputed).

Hence the layer-0 LI membrane is   V0[t,b,o] = sum_n H[n,t] * Y_n[b,o]   with
Y_n = u_n @ w0^T and H = LI-filtered D.  The 32-timestep spike matmul of the
reference collapses to a 4-channel mask matmul: out rows (b,i) = 16*4 = 64
instead of t*b = 512 — 8x fewer MACs, one stream of w0 from HBM (fp8 x64,
DoubleRow, K-accumulated in PSUM; stationary = masks, moving = w0 columns).

  early exit: the reset-free LIF1 membrane is a triple first-order filter of
            V0 with kernel l1-norm <= 50. A cheap certified bound
            max|V0| <= sum_n (max_t|H_n|) |Y_n|  (PE reduction over the 4
            channels) gives 50*bound < 95 (< threshold 100) => layer 1 never
            spikes => layers 2..5 exactly zero => output the zero logit tile.
  slow path: runtime If; V0 materialized exactly from Y by a tiny PE
            expansion against H, then the original per-layer pipeline
            (LIF via scalar_tensor_tensor steps, LI via tensor_tensor_scan
            linear recurrences, bf16 matmuls for w1..w5).
  output:   max over t of V5/10, log_softmax on host (tiny [128,10]).

Internal dtypes: fp8 masks/w0 (x64), bf16 states/Y/weights, fp32 PSUM.
"""

import os
import sys
import numpy as np
import ml_dtypes

import concourse.bass as bass
import concourse.tile as tile
from concourse import bacc, mybir
from concourse.bass_utils import run_bass_kernel_spmd

AL = mybir.AluOpType
F32 = mybir.dt.float32
BF16 = mybir.dt.bfloat16
FP8 = mybir.dt.float8e4
FP8_NP = mybir.dt.np(mybir.dt.float8e4)
BF16_NP = ml_dtypes.bfloat16
W0_SCALE = 64.0

N_CORES = 8
B_CORE = 16
T = 32
NMASK = 4

LAYER_SIZES = [(2000, 12000), (1500, 2000), (1000, 1500), (500, 1000), (100, 500), (10, 100)]
IN_PAD = [12032, 2048, 1536, 1024, 512, 128]
OUT_PAD = [2048, 1536, 1024, 512, 128, 16]
IC = [94, 16, 12, 8, 4, 1]      # input chunks of 128 (contraction)
MC = [16, 12, 8, 4, 1, 1]       # output chunks (M tiles)
M_SIZE = [128, 128, 128, 128, 128, 16]
KP0 = 47                         # layer-0 DoubleRow k-pairs
KSLOT = 12                       # gather slots per partition (max actives)
NFROW = 12160                    # padded feature rows for the gather tables

LAST_EXEC_TIME_NS = None

DT_DECAY_V = np.float32(0.1)   # DT*TAU_MEM_INV
V_TH = np.float32(1.0)


def _enc_first_spike_step(x_scalar):
    """fp32 encoder sim (exactly mirrors reference arithmetic); first spike step or None."""
    f32 = np.float32
    v = f32(0.0)
    x = f32(x_scalar)
    for t in range(T):
        v = f32(v + f32(DT_DECAY_V * f32(-v + x)))
        if f32(v - V_TH) > 0:
            return t
    return None


def _stage0_tables():
    """Host-precomputed structure of the encoder+LIF0 cascade.

    The encoder resets to exactly 0 on spike, so its spike train is periodic
    with period p(x) = 1 + first_spike_step(x); LIF0's response to a period-p
    train is a fixed pattern G[t, p].  The map x -> LIF0-spike-train is
    piecewise constant in x; we compress it to the breakpoints where the
    pattern actually changes and pack patterns as integer codes.
    Returns (breaks [(B_n, delta_n)...], bit_ts [t for each bit, ascending]).
    """
    f32 = np.float32
    # G[t, c]: c = 0 -> silent input; c = p -> period p
    G = np.zeros((T, 34), np.int64)
    for c in range(1, 33):
        v = f32(0.0)
        i = f32(0.0)
        for t in range(T):
            inp = f32(1.0) if (t + 1) % c == 0 else f32(0.0)
            v_dec = f32(v + f32(DT_DECAY_V * f32(-v + i)))
            i_dec = f32(i * f32(0.8))
            z = 1 if f32(v_dec - V_TH) > 0 else 0
            v = f32(0.0) if z else v_dec
            i = f32(i_dec + inp)
            G[t, c] = z
    bit_ts = [t for t in range(T) if G[t].any()]
    code = {c: sum(int(G[ts, c]) << j for j, ts in enumerate(bit_ts)) for c in range(34)}
    code[33] = 0  # period > 32 == silent
    used = [n for n in range(1, 33) if code[n] != code[n + 1]]

    # fp32-exact breakpoints: B_n = min x with first_spike_step <= n-1
    breaks = []
    for n in used:
        lo = np.float32(1.0).view(np.int32)
        hi = np.float32(20.0).view(np.int32)
        while int(hi) - int(lo) > 1:
            mid = np.int32((int(lo) + int(hi)) // 2)
            s = _enc_first_spike_step(mid.view(np.float32))
            if s is not None and s <= n - 1:
                hi = mid
            else:
                lo = mid
        breaks.append((float(np.int32(hi).view(np.float32)), float(code[n] - code[n + 1])))
    return breaks, bit_ts


def _mask_tables():
    """Per-breakpoint spike-train deltas D [4, T] and LI-filtered H [4, T].

    z0[t] (LIF0 spikes) = sum_n (x >= B_n) * D[n, t]  exactly (nested masks).
    V0[t] (scaled LI0 membrane, V = 10*v) = sum_n H[n, t] * Y_n, with
    H the (i' = 0.8 i' + D; V = 0.9 V + i') double filter of D.
    """
    breaks, bit_ts = _stage0_tables()
    assert len(breaks) == NMASK
    deltas = [d for (_, d) in breaks]
    Bs = [b for (b, _) in breaks]
    # Bs descending: passing B_n implies passing all later (smaller) breakpoints.
    csum = np.cumsum(deltas[::-1])[::-1]  # code when masks n..3 are on

    def bits(c):
        c = int(round(c))
        return np.array([(c >> j) & 1 for j in range(len(bit_ts))], np.float64)

    pats = [bits(c) for c in csum] + [np.zeros(len(bit_ts))]
    D = np.zeros((NMASK, T))
    for n in range(NMASK):
        dv = pats[n] - pats[n + 1]
        for j, t in enumerate(bit_ts):
            D[n, t] = dv[j]
    H = np.zeros((NMASK, T))
    for n in range(NMASK):
        ip = 0.0
        V = 0.0
        for t in range(T):
            ip = 0.8 * ip + D[n, t]
            V = 0.9 * V + ip
            H[n, t] = V
    return Bs, D, H


def _install_ntff_hook():
    import types
    if "antenv.axon_hooks" in sys.modules:
        return
    try:
        mod = types.ModuleType("antenv.axon_hooks")
        mod._hook = None
        mod.set_axon_ntff_profile_hook = lambda h: setattr(mod, "_hook", h)
        mod.get_axon_ntff_profile_hook = lambda: mod._hook
        sys.modules["antenv.axon_hooks"] = mod
        from trn_agent_boot.trn_boot import _ntff_profile_via_ctypes
        mod._hook = _ntff_profile_via_ctypes("/opt/axon/libaxon_pjrt.so")
    except Exception:
        pass


def build_body(tc, ctx, nc, xs_ap, w_aps, h_ap, a_ap, out_ap, cv_ap, pio_ap, xg_ap, w0g_ap, dbg_ap=None):
    from contextlib import ExitStack

    Bs, _D, _H = _mask_tables()

    const = ctx.enter_context(tc.tile_pool(name="const", bufs=1))
    psum = ctx.enter_context(tc.tile_pool(name="psum", bufs=8, space="PSUM"))
    ijpool = ctx.enter_context(tc.tile_pool(name="ij", bufs=2))
    spool = ctx.enter_context(tc.tile_pool(name="spikes", bufs=2))

    mask08 = const.tile([128, 512], BF16)
    mask09 = const.tile([128, 512], BF16)

    def emit_masks():
        # decay masks with 0.0 at t=0 of each batch segment (scan segmentation)
        nc.vector.memset(mask08[:], 0.8)
        nc.vector.memset(mask08[:].rearrange("p (b t) -> p b t", b=B_CORE)[:, :, 0:1], 0.0)
        nc.vector.memset(mask09[:], 0.9)
        nc.vector.memset(mask09[:].rearrange("p (b t) -> p b t", b=B_CORE)[:, :, 0:1], 0.0)

    # ---- fast path: per-partition compaction of active features ----
    # A feature (p, c) is active iff any of its 16 images crosses the lowest
    # breakpoint. Per partition there are <= KSLOT active chunks (overflow ->
    # certified fallback to the dense slow path); gather only those w0 rows.
    zero_out = const.tile([M_SIZE[5], B_CORE], F32)
    nc.vector.memset(zero_out[:], 0.0)
    nc.sync.dma_start(out_ap, zero_out[:])

    xr_sb = const.tile([128, 96, B_CORE], F32)
    nc.vector.memset(xr_sb[:, 94:96, :], 0.0)
    nc.sync.dma_start(xr_sb[:, :94, :].rearrange("p (a j) b -> p a j b", j=2), xs_ap)
    cv = const.tile([128, KP0 * 2], F32)       # c+1 per (kp, j), const
    nc.sync.dma_start(cv[:], cv_ap)
    pio = const.tile([128, B_CORE], F32)       # partition index, const
    nc.sync.dma_start(pio[:], pio_ap)
    hc = const.tile([NMASK, T], BF16)      # H for the Else expansion
    nc.sync.dma_start(hc[:], h_ap)
    ac = const.tile([B_CORE * NMASK, B_CORE], BF16)  # bound-reduction matrix
    nc.sync.dma_start(ac[:], a_ap)

    Ysb = const.tile([B_CORE * NMASK, 4, 512], BF16)    # descaled Y (Else only)
    absY = const.tile([B_CORE * NMASK, 4, 512], BF16)   # |Y| for the bound

    xm = const.tile([128, KP0 * 2], F32)
    nc.vector.tensor_reduce(xm[:], xr_sb[:, :94, :], mybir.AxisListType.X, AL.max)
    act = const.tile([128, KP0 * 2], F32)
    nc.vector.tensor_scalar(act[:], xm[:], float(Bs[NMASK - 1]), None, AL.is_ge)
    ones94 = const.tile([128, KP0 * 2], F32)
    nc.vector.memset(ones94[:], 1.0)
    incl = const.tile([128, KP0 * 2], F32)
    nc.vector.tensor_tensor_scan(incl[:], ones94[:], act[:], 0.0, AL.mult, AL.add)
    excl = const.tile([128, KP0 * 2], F32)
    nc.vector.tensor_tensor(excl[:], incl[:], act[:], AL.subtract)
    acv = const.tile([128, KP0 * 2], F32)   # act * (c+1)
    nc.vector.tensor_tensor(acv[:], act[:], cv[:], AL.mult)

    # ck[p, k] = chunk index of k-th active chunk of partition p, or 94 (pad)
    ck = const.tile([128, KSLOT], F32)
    tmpa = const.tile([128, KSLOT, KP0 * 2], F32)
    for k in range(KSLOT):
        nc.vector.scalar_tensor_tensor(
            tmpa[:, k, :], excl[:], float(k), acv[:], AL.is_equal, AL.mult
        )
    nc.vector.tensor_reduce(ck[:], tmpa[:], mybir.AxisListType.X, AL.add)
    tmpk = const.tile([128, KSLOT], F32)
    nc.vector.tensor_scalar(tmpk[:], ck[:], 0.0, 95.0, AL.is_equal, AL.mult)
    nc.vector.tensor_tensor(ck[:], ck[:], tmpk[:], AL.add)
    nc.vector.tensor_scalar(ck[:], ck[:], 1.0, None, AL.subtract)

    fof = const.tile([128, KSLOT], F32)     # row index = c*128 + p
    nc.vector.scalar_tensor_tensor(fof[:], ck[:], 128.0, pio[:, :KSLOT], AL.mult, AL.add)
    foi = const.tile([128, KSLOT], mybir.dt.int32)
    nc.vector.tensor_copy(foi[:], fof[:])

    wg = const.tile([128, KSLOT, 2048], FP8)
    for k in range(KSLOT):
        nc.gpsimd.indirect_dma_start(
            out=wg[:, k, :], out_offset=None, in_=w0g_ap,
            in_offset=bass.IndirectOffsetOnAxis(ap=foi[:, k:k + 1], axis=0),
        )
    xg = const.tile([128, KSLOT, B_CORE], F32)
    for k in range(KSLOT):
        nc.gpsimd.indirect_dma_start(
            out=xg[:, k, :], out_offset=None, in_=xg_ap,
            in_offset=bass.IndirectOffsetOnAxis(ap=foi[:, k:k + 1], axis=0),
        )

    uc = const.tile([128, KSLOT // 2, 2, B_CORE, NMASK], FP8)
    xgv = xg[:].rearrange("p (t j) b -> p t j b", j=2)
    for i, bn in enumerate(Bs):
        nc.vector.tensor_scalar(uc[:, :, :, :, i], xgv, float(bn), None, AL.is_ge)

    wgv = wg[:].rearrange("p (t j) o -> p t j o", j=2)
    ps = [psum.tile([B_CORE * NMASK, 512], F32, name=f"ps{og}", bufs=1) for og in range(4)]
    for t in range(KSLOT // 2):
        for og in range(4):
            nc.tensor.matmul(
                ps[og][:],
                uc[:, t, :, :, :],
                wgv[:, t, :, og * 512:(og + 1) * 512],
                start=(t == 0),
                stop=(t == KSLOT // 2 - 1),
                perf_mode=mybir.MatmulPerfMode.DoubleRow,
            )
    for og in range(4):
        nc.scalar.activation(
            absY[:, og, :], ps[og][:], mybir.ActivationFunctionType.Abs,
            scale=1.0 / W0_SCALE,
        )
    # R[b, o] = sum_i A_i |Y_i[b, o]| via PE: stationary = A [(b,i), b'], then
    # 50 * max R < 95 certifies "layer 1 never spikes".
    rmx = const.tile([B_CORE, 4], F32)
    for og in range(4):
        psr = psum.tile([B_CORE, 512], F32, bufs=1)
        nc.tensor.matmul(psr[:], ac[:], absY[:, og, :], start=True, stop=True)
        nc.vector.tensor_reduce(rmx[:, og:og + 1], psr[:], mybir.AxisListType.X, AL.max)

    from concourse import bass_isa
    red2 = const.tile([128, 2], F32)
    nc.vector.memset(red2[:], 0.0)
    nc.vector.tensor_reduce(red2[0:B_CORE, 0:1], rmx[:], mybir.AxisListType.X, AL.max)
    nc.vector.tensor_copy(red2[:, 1:2], incl[:, KP0 * 2 - 1:])
    gred = const.tile([128, 2], F32)
    nc.gpsimd.partition_all_reduce(gred[:], red2[:], 128, bass_isa.ReduceOp.max)
    # slot overflow (max_p cnt > KSLOT) forces the dense slow path: +1000
    ovf = const.tile([1, 1], F32)
    nc.vector.tensor_scalar(ovf[:], gred[0:1, 1:2], float(KSLOT), 1000.0, AL.is_gt, AL.mult)
    gmax_s = const.tile([1, 1], F32)
    nc.vector.scalar_tensor_tensor(gmax_s[:], gred[0:1, 0:1], 50.0, ovf[:], AL.mult, AL.add)
    gmax_i = const.tile([1, 1], mybir.dt.int32)
    nc.vector.tensor_copy(gmax_i[:], gmax_s[:])
    if dbg_ap is not None:
        dbg = const.tile([B_CORE, 8], F32)
        nc.vector.memset(dbg[:], 0.0)
        nc.vector.tensor_copy(dbg[:, 0:4], rmx[:])
        nc.vector.tensor_copy(dbg[:, 4:6], gred[0:B_CORE, :])
        nc.vector.tensor_copy(dbg[:, 6:7], ck[0:B_CORE, 0:1])
        nc.sync.dma_start(dbg_ap, dbg[:])
    _, (sval,) = nc.values_load_multi_w_load_instructions(
        gmax_i[0:1, 0:1], skip_runtime_bounds_check=True
    )

    def emit_dense_Y():
        # exact dense recomputation of Y (covers slot overflow), Else only
        with ExitStack() as phd:
            pd = phd.enter_context(tc.tile_pool(name="dense0", bufs=1))
            uf = pd.tile([128, KP0, 2, B_CORE, NMASK], FP8, tag="uf")
            xrv = xr_sb[:, :94, :].rearrange("p (a j) b -> p a j b", j=2)
            for i, bn in enumerate(Bs):
                nc.vector.tensor_scalar(uf[:, :, :, :, i], xrv, float(bn), None, AL.is_ge)
            w0pool = phd.enter_context(tc.tile_pool(name="w0s", bufs=2))
            W0_GROUPS = [(0, 2), (2, 10), (10, 18), (18, 26), (26, 34), (34, 42), (42, 47)]
            psd = [psum.tile([B_CORE * NMASK, 512], F32, name=f"ps{og}", bufs=1) for og in range(4)]
            for g0, g1 in W0_GROUPS:
                wt = w0pool.tile([128, 8, 2, 2048], FP8, name="wt")
                nc.sync.dma_start(
                    wt[:, :g1 - g0, :, :],
                    w_aps[0][g0:g1].rearrange("g p j o -> p g j o"),
                )
                for kp in range(g0, g1):
                    for og in range(4):
                        nc.tensor.matmul(
                            psd[og][:],
                            uf[:, kp, :, :, :],
                            wt[:, kp - g0, :, og * 512:(og + 1) * 512],
                            start=(kp == 0),
                            stop=(kp == KP0 - 1),
                            perf_mode=mybir.MatmulPerfMode.DoubleRow,
                        )
            for og in range(4):
                nc.vector.tensor_scalar(
                    Ysb[:, og, :], psd[og][:], 1.0 / W0_SCALE, None, AL.mult
                )

    # ---- slow-path helpers (baseline per-layer pipeline) ----
    spikes = None  # current layer's input spike tensor, [128, IC[k], 16, 32] bf16

    def lif_phase(k, V, pk):
        nonlocal spikes
        C = MC[k]
        Vv = V[:].rearrange("p m (b t) -> p m b t", t=T)
        S = spool.tile([128, C, B_CORE, T], BF16, tag="S")
        P = pk.tile([128, C, B_CORE], BF16, tag="P")
        Q = pk.tile([128, C, B_CORE], BF16, tag="Q")
        nc.vector.memset(P[:], 0.0)
        nc.vector.memset(Q[:], 0.0)
        for t in range(T):
            nc.vector.scalar_tensor_tensor(P[:], P[:], 0.9, Q[:], AL.mult, AL.add)
            nc.vector.tensor_scalar(S[:, :, :, t], P[:], 100.0, None, AL.is_gt)
            nc.vector.scalar_tensor_tensor(P[:], P[:], 100.0, P[:], AL.is_le, AL.mult)
            nc.vector.scalar_tensor_tensor(Q[:], Q[:], 0.8, Vv[:, :, :, t], AL.mult, AL.add)
        spikes = S

    def layer_phase(k):
        nonlocal spikes
        M = M_SIZE[k]
        with ExitStack() as ph:
            pk = ph.enter_context(tc.tile_pool(name=f"phase{k + 1}", bufs=1))
            if k == 5:
                V = pk.tile([M, 512], F32, tag="V5")
            else:
                V = pk.tile([128, MC[k], 512], BF16, tag=f"V{k}")

            wk_sb = pk.tile([128, IC[k], OUT_PAD[k]], BF16, tag=f"w{k}")
            nc.sync.dma_start(wk_sb[:], w_aps[k])

            for m in range(MC[k]):
                ps = psum.tile([128, 512], F32, bufs=2)
                for kc in range(IC[k]):
                    nc.tensor.matmul(
                        ps[:M, :],
                        wk_sb[:, kc, m * 128:m * 128 + M],
                        spikes[:, kc, :, :],
                        start=(kc == 0),
                        stop=(kc == IC[k] - 1),
                    )
                j_src = ps[:M, :]
                ij = ijpool.tile([128, 512], BF16)
                nc.vector.tensor_tensor_scan(ij[:M, :], mask08[:M, :], j_src, 0.0, AL.mult, AL.add)
                if k == 5:
                    nc.vector.tensor_tensor_scan(V[:, :], mask09[:M, :], ij[:M, :], 0.0, AL.mult, AL.add)
                else:
                    nc.vector.tensor_tensor_scan(V[:, m, :], mask09[:, :], ij[:, :], 0.0, AL.mult, AL.add)

            if k == 5:
                rmax = pk.tile([M, B_CORE], F32)
                nc.vector.tensor_reduce(
                    rmax[:], V[:].rearrange("p (b t) -> p b t", b=B_CORE),
                    mybir.AxisListType.X, AL.max,
                )
                nc.sync.dma_start(out_ap, rmax[:])
            else:
                lif_phase(k, V, pk)

    with tc.If(sval < 95) as cmp:
        pass
    with cmp.Else():
        emit_masks()
        emit_dense_Y()
        with ExitStack() as phl:
            pl = phl.enter_context(tc.tile_pool(name="lif1", bufs=1))
            # transpose Y to partitions = i for PE expansion against H
            Yt = pl.tile([NMASK, B_CORE, 4, 512], BF16, tag="Yt")
            for b in range(B_CORE):
                nc.sync.dma_start(
                    Yt[:, b, :, :], Ysb[b * NMASK:(b + 1) * NMASK, :, :]
                )
            V0 = pl.tile([128, MC[0], 512], BF16, tag="V0")
            for m in range(MC[0]):
                psv = psum.tile([128, 512], F32, bufs=1)
                for b in range(B_CORE):
                    nc.tensor.matmul(
                        psv[:, b * T:(b + 1) * T],
                        Yt[:, b, m // 4, (m % 4) * 128:(m % 4) * 128 + 128],
                        hc[:],
                        start=True, stop=True,
                    )
                nc.scalar.activation(
                    V0[:, m, :], psv[:], mybir.ActivationFunctionType.Copy, scale=1.0
                )
            lif_phase(0, V0, pl)
        for k in range(1, 6):
            layer_phase(k)


def build_nc():
    from contextlib import ExitStack

    nc = bacc.Bacc("TRN2", debug=False, num_devices=N_CORES)
    xs = nc.dram_tensor("xs", [128, KP0, 2, B_CORE], F32, kind="ExternalInput")
    w_t = [nc.dram_tensor("w0t", [KP0, 128, 2, 2048], FP8, kind="ExternalInput")]
    for k in range(1, 6):
        w_t.append(
            nc.dram_tensor(f"w{k}t", [128, IC[k], OUT_PAD[k]], BF16, kind="ExternalInput")
        )
    hconst = nc.dram_tensor("hconst", [NMASK, T], BF16, kind="ExternalInput")
    aconst = nc.dram_tensor("aconst", [B_CORE * NMASK, B_CORE], BF16, kind="ExternalInput")
    cvconst = nc.dram_tensor("cvconst", [128, KP0 * 2], F32, kind="ExternalInput")
    pioconst = nc.dram_tensor("pioconst", [128, B_CORE], F32, kind="ExternalInput")
    xgath = nc.dram_tensor("xgath", [NFROW, B_CORE], F32, kind="ExternalInput")
    w0gath = nc.dram_tensor("w0gath", [NFROW, 2048], FP8, kind="ExternalInput")
    out = nc.dram_tensor("out", [M_SIZE[5], B_CORE], F32, kind="ExternalOutput")
    dbg = nc.dram_tensor("dbg", [B_CORE, 8], F32, kind="ExternalOutput")

    with tile.TileContext(nc) as tc, ExitStack() as ctx:
        build_body(tc, ctx, nc, xs.ap(), [w.ap() for w in w_t],
                   hconst.ap(), aconst.ap(), out.ap(),
                   cvconst.ap(), pioconst.ap(), xgath.ap(), w0gath.ap(),
                   dbg_ap=dbg.ap())
    nc.compile()
    return nc


def prep_inputs(images, ws):
    """Host-side marshalling: pad/transpose/cast weights, rearrange images."""
    x = np.asarray(images).reshape(128, -1).astype(np.float32)  # [B, 12000]
    xs = np.zeros((128, 12032), np.float32)
    xs[:, :12000] = x
    # [p, kp, j, b] with feature f = (2*kp + j)*128 + p
    xs_r = xs.reshape(128, 47, 2, 128).transpose(3, 1, 2, 0)  # [128p, 47, 2, 128b]
    xs_cores = [
        np.ascontiguousarray(xs_r[:, :, :, c * B_CORE:(c + 1) * B_CORE])
        for c in range(N_CORES)
    ]

    w_prepped = []
    wT0 = np.zeros((12032, 2048), np.float32)
    wT0[:12000, :2000] = np.asarray(ws[0]).T * np.float32(W0_SCALE)
    # [47 kp, 128 p, 2 j, 2048 o]: feature f = (2*kp + j)*128 + p
    w0p = wT0.reshape(47, 2, 128, 2048).transpose(0, 2, 1, 3)
    w_prepped.append(np.ascontiguousarray(w0p.astype(FP8_NP)))
    for k in range(1, 6):
        out_f, in_f = LAYER_SIZES[k]
        wTk = np.zeros((IN_PAD[k], OUT_PAD[k]), np.float32)
        wTk[:in_f, :out_f] = np.asarray(ws[k]).T
        wkp = wTk.reshape(IC[k], 128, OUT_PAD[k]).transpose(1, 0, 2)  # [128p, IC, OUT]
        w_prepped.append(np.ascontiguousarray(wkp.astype(BF16_NP)))

    xgath_cores = []
    for c in range(N_CORES):
        xgc = np.zeros((NFROW, B_CORE), np.float32)
        xgc[:12032] = xs[c * B_CORE:(c + 1) * B_CORE].T
        xgath_cores.append(np.ascontiguousarray(xgc))
    w0gath = np.zeros((NFROW, 2048), np.float32)
    w0gath[:12032] = wT0
    w0gath = np.ascontiguousarray(w0gath.astype(FP8_NP))
    cvmat = np.ascontiguousarray(
        np.tile(np.arange(1, KP0 * 2 + 1, dtype=np.float32)[None, :], (128, 1))
    )
    piomat = np.ascontiguousarray(
        np.tile(np.arange(128, dtype=np.float32)[:, None], (1, B_CORE))
    )
    _Bs, _D, H = _mask_tables()
    hmat = np.ascontiguousarray(H.astype(BF16_NP))  # [4, 32]
    A = np.abs(H).max(1)  # per-channel max_t |H|
    amat = np.zeros((B_CORE * NMASK, B_CORE), np.float32)
    for b in range(B_CORE):
        for i in range(NMASK):
            amat[b * NMASK + i, b] = A[i]
    amat = np.ascontiguousarray(amat.astype(BF16_NP))
    return xs_cores, w_prepped, hmat, amat, cvmat, piomat, xgath_cores, w0gath


_NC_CACHE = {}


def kernel(images, w0, w1, w2, w3, w4, w5):
    global LAST_EXEC_TIME_NS
    ws = [w0, w1, w2, w3, w4, w5]
    (xs_cores, w_prepped, hmat, amat, cvmat, piomat,
     xgath_cores, w0gath) = prep_inputs(images, ws)

    trace = os.environ.get("KERNEL_TRACE", "0") == "1"
    if trace:
        _install_ntff_hook()

    if "nc" not in _NC_CACHE:
        _NC_CACHE["nc"] = build_nc()
    nc = _NC_CACHE["nc"]

    in_maps = []
    for c in range(N_CORES):
        m = {"xs": xs_cores[c], "w0t": w_prepped[0], "hconst": hmat, "aconst": amat,
             "cvconst": cvmat, "pioconst": piomat, "xgath": xgath_cores[c],
             "w0gath": w0gath}
        for k in range(1, 6):
            m[f"w{k}t"] = w_prepped[k]
        in_maps.append(m)

    res = run_bass_kernel_spmd(
        nc, in_maps, core_ids=list(range(N_CORES)), trace=trace
    )
    LAST_EXEC_TIME_NS = res.exec_time_ns
    _NC_CACHE["res"] = res

    # out[c] is [16 feats, 16 batch]; valid feats :10; logits = max_t(V5)/10
    logits = np.concatenate(
        [np.asarray(res.results[c]["out"])[:10, :].T for c in range(N_CORES)], axis=0
    ).astype(np.float32) / np.float32(10.0)
    mx = logits.max(axis=1, keepdims=True)
    sh = logits - mx
    out = sh - np.log(np.exp(sh).sum(axis=1, keepdims=True))
    return out.astype(np.float32)


# revision 18
# speedup vs baseline: 3.9686x; 1.1882x over previous
"""Trainium2 Bass kernel for nn_CaptchaRecognizer (norse-style SNN).

Strategy (pure data-parallel over batch, 8 NeuronCores, 16 images each):

The encoder resets to exactly 0 on spike, so the encoder+LIF0 cascade is a
piecewise-constant function of x alone: only 4 fp32-exact breakpoints B_n
matter, and the LIF0 spike train is EXACTLY LINEAR in the 4 nested threshold
masks u_n = (x >= B_n):   z0[t] = sum_n D[n,t] * u_n   (D host-precomputed).

Hence the layer-0 LI membrane is   V0[t,b,o] = sum_n H[n,t] * Y_n[b,o]   with
Y_n = u_n @ w0^T and H = LI-filtered D.  The 32-timestep spike matmul of the
reference collapses to a 4-channel mask matmul: out rows (b,i) = 16*4 = 64
instead of t*b = 512 — 8x fewer MACs, one stream of w0 from HBM (fp8 x64,
DoubleRow, K-accumulated in PSUM; stationary = masks, moving = w0 columns).

  early exit: the reset-free LIF1 membrane is a triple first-order filter of
            V0 with kernel l1-norm <= 50. A cheap certified bound
            max|V0| <= sum_n (max_t|H_n|) |Y_n|  (PE reduction over the 4
            channels) gives 50*bound < 95 (< threshold 100) => layer 1 never
            spikes => layers 2..5 exactly zero => output the zero logit tile.
  slow path: runtime If; V0 materialized exactly from Y by a tiny PE
            expansion against H, then the original per-layer pipeline
            (LIF via scalar_tensor_tensor steps, LI via tensor_tensor_scan
            linear recurrences, bf16 matmuls for w1..w5).
  output:   max over t of V5/10, log_softmax on host (tiny [128,10]).

Internal dtypes: fp8 masks/w0 (x64), bf16 states/Y/weights, fp32 PSUM.
"""

import os
import sys
import numpy as np
import ml_dtypes

import concourse.bass as bass
import concourse.tile as tile
from concourse import bacc, mybir
from concourse.bass_utils import run_bass_kernel_spmd

AL = mybir.AluOpType
F32 = mybir.dt.float32
BF16 = mybir.dt.bfloat16
FP8 = mybir.dt.float8e4
FP8_NP = mybir.dt.np(mybir.dt.float8e4)
BF16_NP = ml_dtypes.bfloat16
W0_SCALE = 64.0

N_CORES = 8
B_CORE = 16
T = 32
NMASK = 4

LAYER_SIZES = [(2000, 12000), (1500, 2000), (1000, 1500), (500, 1000), (100, 500), (10, 100)]
IN_PAD = [12032, 2048, 1536, 1024, 512, 128]
OUT_PAD = [2048, 1536, 1024, 512, 128, 16]
IC = [94, 16, 12, 8, 4, 1]      # input chunks of 128 (contraction)
MC = [16, 12, 8, 4, 1, 1]       # output chunks (M tiles)
M_SIZE = [128, 128, 128, 128, 128, 16]
KP0 = 47                         # layer-0 DoubleRow k-pairs
KSLOT = 10                       # gather slots per partition (max actives)
NFROW = 12160                    # padded feature rows for the gather tables

LAST_EXEC_TIME_NS = None

DT_DECAY_V = np.float32(0.1)   # DT*TAU_MEM_INV
V_TH = np.float32(1.0)


def _enc_first_spike_step(x_scalar):
    """fp32 encoder sim (exactly mirrors reference arithmetic); first spike step or None."""
    f32 = np.float32
    v = f32(0.0)
    x = f32(x_scalar)
    for t in range(T):
        v = f32(v + f32(DT_DECAY_V * f32(-v + x)))
        if f32(v - V_TH) > 0:
            return t
    return None


def _stage0_tables():
    """Host-precomputed structure of the encoder+LIF0 cascade.

    The encoder resets to exactly 0 on spike, so its spike train is periodic
    with period p(x) = 1 + first_spike_step(x); LIF0's response to a period-p
    train is a fixed pattern G[t, p].  The map x -> LIF0-spike-train is
    piecewise constant in x; we compress it to the breakpoints where the
    pattern actually changes and pack patterns as integer codes.
    Returns (breaks [(B_n, delta_n)...], bit_ts [t for each bit, ascending]).
    """
    f32 = np.float32
    # G[t, c]: c = 0 -> silent input; c = p -> period p
    G = np.zeros((T, 34), np.int64)
    for c in range(1, 33):
        v = f32(0.0)
        i = f32(0.0)
        for t in range(T):
            inp = f32(1.0) if (t + 1) % c == 0 else f32(0.0)
            v_dec = f32(v + f32(DT_DECAY_V * f32(-v + i)))
            i_dec = f32(i * f32(0.8))
            z = 1 if f32(v_dec - V_TH) > 0 else 0
            v = f32(0.0) if z else v_dec
            i = f32(i_dec + inp)
            G[t, c] = z
    bit_ts = [t for t in range(T) if G[t].any()]
    code = {c: sum(int(G[ts, c]) << j for j, ts in enumerate(bit_ts)) for c in range(34)}
    code[33] = 0  # period > 32 == silent
    used = [n for n in range(1, 33) if code[n] != code[n + 1]]

    # fp32-exact breakpoints: B_n = min x with first_spike_step <= n-1
    breaks = []
    for n in used:
        lo = np.float32(1.0).view(np.int32)
        hi = np.float32(20.0).view(np.int32)
        while int(hi) - int(lo) > 1:
            mid = np.int32((int(lo) + int(hi)) // 2)
            s = _enc_first_spike_step(mid.view(np.float32))
            if s is not None and s <= n - 1:
                hi = mid
            else:
                lo = mid
        breaks.append((float(np.int32(hi).view(np.float32)), float(code[n] - code[n + 1])))
    return breaks, bit_ts


def _mask_tables():
    """Per-breakpoint spike-train deltas D [4, T] and LI-filtered H [4, T].

    z0[t] (LIF0 spikes) = sum_n (x >= B_n) * D[n, t]  exactly (nested masks).
    V0[t] (scaled LI0 membrane, V = 10*v) = sum_n H[n, t] * Y_n, with
    H the (i' = 0.8 i' + D; V = 0.9 V + i') double filter of D.
    """
    breaks, bit_ts = _stage0_tables()
    assert len(breaks) == NMASK
    deltas = [d for (_, d) in breaks]
    Bs = [b for (b, _) in breaks]
    # Bs descending: passing B_n implies passing all later (smaller) breakpoints.
    csum = np.cumsum(deltas[::-1])[::-1]  # code when masks n..3 are on

    def bits(c):
        c = int(round(c))
        return np.array([(c >> j) & 1 for j in range(len(bit_ts))], np.float64)

    pats = [bits(c) for c in csum] + [np.zeros(len(bit_ts))]
    D = np.zeros((NMASK, T))
    for n in range(NMASK):
        dv = pats[n] - pats[n + 1]
        for j, t in enumerate(bit_ts):
            D[n, t] = dv[j]
    H = np.zeros((NMASK, T))
    for n in range(NMASK):
        ip = 0.0
        V = 0.0
        for t in range(T):
            ip = 0.8 * ip + D[n, t]
            V = 0.9 * V + ip
            H[n, t] = V
    return Bs, D, H


def _install_ntff_hook():
    import types
    if "antenv.axon_hooks" in sys.modules:
        return
    try:
        mod = types.ModuleType("antenv.axon_hooks")
        mod._hook = None
        mod.set_axon_ntff_profile_hook = lambda h: setattr(mod, "_hook", h)
        mod.get_axon_ntff_profile_hook = lambda: mod._hook
        sys.modules["antenv.axon_hooks"] = mod
        from trn_agent_boot.trn_boot import _ntff_profile_via_ctypes
        mod._hook = _ntff_profile_via_ctypes("/opt/axon/libaxon_pjrt.so")
    except Exception:
        pass


def build_body(tc, ctx, nc, xs_ap, w_aps, h_ap, a_ap, out_ap, cv_ap, pio_ap, xg_ap, w0g_ap, dbg_ap=None):
    from contextlib import ExitStack

    Bs, _D, _H = _mask_tables()

    const = ctx.enter_context(tc.tile_pool(name="const", bufs=1))
    psum = ctx.enter_context(tc.tile_pool(name="psum", bufs=8, space="PSUM"))
    ijpool = ctx.enter_context(tc.tile_pool(name="ij", bufs=2))
    spool = ctx.enter_context(tc.tile_pool(name="spikes", bufs=2))

    mask08 = const.tile([128, 512], BF16)
    mask09 = const.tile([128, 512], BF16)

    def emit_masks():
        # decay masks with 0.0 at t=0 of each batch segment (scan segmentation)
        nc.vector.memset(mask08[:], 0.8)
        nc.vector.memset(mask08[:].rearrange("p (b t) -> p b t", b=B_CORE)[:, :, 0:1], 0.0)
        nc.vector.memset(mask09[:], 0.9)
        nc.vector.memset(mask09[:].rearrange("p (b t) -> p b t", b=B_CORE)[:, :, 0:1], 0.0)

    # ---- fast path: per-partition compaction of active features ----
    # A feature (p, c) is active iff any of its 16 images crosses the lowest
    # breakpoint. Per partition there are <= KSLOT active chunks (overflow ->
    # certified fallback to the dense slow path); gather only those w0 rows.
    zero_out = const.tile([M_SIZE[5], B_CORE], F32)
    nc.vector.memset(zero_out[:], 0.0)
    nc.sync.dma_start(out_ap, zero_out[:])

    xr_sb = const.tile([128, 96, B_CORE], F32)
    nc.vector.memset(xr_sb[:, 94:96, :], 0.0)
    nc.sync.dma_start(xr_sb[:, :94, :].rearrange("p (a j) b -> p a j b", j=2), xs_ap)
    cv = const.tile([128, KP0 * 2], F32)       # c+1 per (kp, j), const
    nc.sync.dma_start(cv[:], cv_ap)
    pio = const.tile([128, B_CORE], F32)       # partition index, const
    nc.sync.dma_start(pio[:], pio_ap)
    hc = const.tile([NMASK, T], BF16)      # H for the Else expansion
    nc.sync.dma_start(hc[:], h_ap)
    ac = const.tile([B_CORE * NMASK, B_CORE], BF16)  # bound-reduction matrix
    nc.sync.dma_start(ac[:], a_ap)

    Ysb = const.tile([B_CORE * NMASK, 4, 512], BF16)    # descaled Y (Else only)
    absY = const.tile([B_CORE * NMASK, 4, 512], BF16)   # |Y| for the bound

    xm = const.tile([128, KP0 * 2], F32)
    nc.vector.tensor_reduce(xm[:], xr_sb[:, :94, :], mybir.AxisListType.X, AL.max)
    act = const.tile([128, KP0 * 2], F32)
    nc.vector.tensor_scalar(act[:], xm[:], float(Bs[NMASK - 1]), None, AL.is_ge)
    ones94 = const.tile([128, KP0 * 2], F32)
    nc.vector.memset(ones94[:], 1.0)
    incl = const.tile([128, KP0 * 2], F32)
    nc.vector.tensor_tensor_scan(incl[:], ones94[:], act[:], 0.0, AL.mult, AL.add)
    excl = const.tile([128, KP0 * 2], F32)
    nc.vector.tensor_tensor(excl[:], incl[:], act[:], AL.subtract)
    acv = const.tile([128, KP0 * 2], F32)   # act * (c+1)
    nc.vector.tensor_tensor(acv[:], act[:], cv[:], AL.mult)

    # ck[p, k] = chunk index of k-th active chunk of partition p, or 94 (pad)
    ck = const.tile([128, KSLOT], F32)
    tmpa = const.tile([128, KSLOT, KP0 * 2], F32)
    for k in range(KSLOT):
        nc.vector.scalar_tensor_tensor(
            tmpa[:, k, :], excl[:], float(k), acv[:], AL.is_equal, AL.mult
        )
    nc.vector.tensor_reduce(ck[:], tmpa[:], mybir.AxisListType.X, AL.add)
    tmpk = const.tile([128, KSLOT], F32)
    nc.vector.tensor_scalar(tmpk[:], ck[:], 0.0, 95.0, AL.is_equal, AL.mult)
    nc.vector.tensor_tensor(ck[:], ck[:], tmpk[:], AL.add)
    nc.vector.tensor_scalar(ck[:], ck[:], 1.0, None, AL.subtract)

    fof = const.tile([128, KSLOT], F32)     # row index = c*128 + p
    nc.vector.scalar_tensor_tensor(fof[:], ck[:], 128.0, pio[:, :KSLOT], AL.mult, AL.add)
    foi = const.tile([128, KSLOT], mybir.dt.int32)
    nc.vector.tensor_copy(foi[:], fof[:])

    wg = const.tile([128, KSLOT, 2048], FP8)
    for k in range(KSLOT):
        nc.gpsimd.indirect_dma_start(
            out=wg[:, k, :], out_offset=None, in_=w0g_ap,
            in_offset=bass.IndirectOffsetOnAxis(ap=foi[:, k:k + 1], axis=0),
        )
    xg = const.tile([128, KSLOT, B_CORE], F32)
    for k in range(KSLOT):
        nc.gpsimd.indirect_dma_start(
            out=xg[:, k, :], out_offset=None, in_=xg_ap,
            in_offset=bass.IndirectOffsetOnAxis(ap=foi[:, k:k + 1], axis=0),
        )

    uc = const.tile([128, KSLOT // 2, 2, B_CORE, NMASK], FP8)
    xgv = xg[:].rearrange("p (t j) b -> p t j b", j=2)
    wgv = wg[:].rearrange("p (t j) o -> p t j o", j=2)
    ps = [psum.tile([B_CORE * NMASK, 512], F32, name=f"ps{og}", bufs=1) for og in range(4)]
    for t in range(KSLOT // 2):
        for i, bn in enumerate(Bs):
            nc.vector.tensor_scalar(
                uc[:, t, :, :, i], xgv[:, t, :, :], float(bn), None, AL.is_ge
            )
        for og in range(4):
            nc.tensor.matmul(
                ps[og][:],
                uc[:, t, :, :, :],
                wgv[:, t, :, og * 512:(og + 1) * 512],
                start=(t == 0),
                stop=(t == KSLOT // 2 - 1),
                perf_mode=mybir.MatmulPerfMode.DoubleRow,
            )
    for og in range(4):
        nc.scalar.activation(
            absY[:, og, :], ps[og][:], mybir.ActivationFunctionType.Abs,
            scale=1.0 / W0_SCALE,
        )
    # R[b, o] = sum_i A_i |Y_i[b, o]| via PE: stationary = A [(b,i), b'], then
    # 50 * max R < 95 certifies "layer 1 never spikes".
    rmx = const.tile([B_CORE, 4], F32)
    for og in range(4):
        psr = psum.tile([B_CORE, 512], F32, bufs=1)
        nc.tensor.matmul(psr[:], ac[:], absY[:, og, :], start=True, stop=True)
        nc.vector.tensor_reduce(rmx[:, og:og + 1], psr[:], mybir.AxisListType.X, AL.max)

    from concourse import bass_isa
    red2 = const.tile([128, 2], F32)
    nc.vector.memset(red2[:], 0.0)
    nc.vector.tensor_reduce(red2[0:B_CORE, 0:1], rmx[:], mybir.AxisListType.X, AL.max)
    nc.vector.tensor_copy(red2[:, 1:2], incl[:, KP0 * 2 - 1:])
    gred = const.tile([128, 2], F32)
    nc.gpsimd.partition_all_reduce(gred[:], red2[:], 128, bass_isa.ReduceOp.max)
    # slot overflow (max_p cnt > KSLOT) forces the dense slow path: +1000
    ovf = const.tile([1, 1], F32)
    nc.vector.tensor_scalar(ovf[:], gred[0:1, 1:2], float(KSLOT), 1000.0, AL.is_gt, AL.mult)
    gmax_s = const.tile([1, 1], F32)
    nc.vector.scalar_tensor_tensor(gmax_s[:], gred[0:1, 0:1], 50.0, ovf[:], AL.mult, AL.add)
    gmax_i = const.tile([1, 1], mybir.dt.int32)
    nc.vector.tensor_copy(gmax_i[:], gmax_s[:])
    if dbg_ap is not None:
        dbg = const.tile([B_CORE, 8], F32)
        nc.vector.memset(dbg[:], 0.0)
        nc.vector.tensor_copy(dbg[:, 0:4], rmx[:])
        nc.vector.tensor_copy(dbg[:, 4:6], gred[0:B_CORE, :])
        nc.vector.tensor_copy(dbg[:, 6:7], ck[0:B_CORE, 0:1])
        nc.sync.dma_start(dbg_ap, dbg[:])
    _, (sval,) = nc.values_load_multi_w_load_instructions(
        gmax_i[0:1, 0:1], skip_runtime_bounds_check=True
    )

    def emit_dense_Y():
        # exact dense recomputation of Y (covers slot overflow), Else only
        with ExitStack() as phd:
            pd = phd.enter_context(tc.tile_pool(name="dense0", bufs=1))
            uf = pd.tile([128, KP0, 2, B_CORE, NMASK], FP8, tag="uf")
            xrv = xr_sb[:, :94, :].rearrange("p (a j) b -> p a j b", j=2)
            for i, bn in enumerate(Bs):
                nc.vector.tensor_scalar(uf[:, :, :, :, i], xrv, float(bn), None, AL.is_ge)
            w0pool = phd.enter_context(tc.tile_pool(name="w0s", bufs=2))
            W0_GROUPS = [(0, 2), (2, 10), (10, 18), (18, 26), (26, 34), (34, 42), (42, 47)]
            psd = [psum.tile([B_CORE * NMASK, 512], F32, name=f"ps{og}", bufs=1) for og in range(4)]
            for g0, g1 in W0_GROUPS:
                wt = w0pool.tile([128, 8, 2, 2048], FP8, name="wt")
                nc.sync.dma_start(
                    wt[:, :g1 - g0, :, :],
                    w_aps[0][g0:g1].rearrange("g p j o -> p g j o"),
                )
                for kp in range(g0, g1):
                    for og in range(4):
                        nc.tensor.matmul(
                            psd[og][:],
                            uf[:, kp, :, :, :],
                            wt[:, kp - g0, :, og * 512:(og + 1) * 512],
                            start=(kp == 0),
                            stop=(kp == KP0 - 1),
                            perf_mode=mybir.MatmulPerfMode.DoubleRow,
                        )
            for og in range(4):
                nc.vector.tensor_scalar(
                    Ysb[:, og, :], psd[og][:], 1.0 / W0_SCALE, None, AL.mult
                )

    # ---- slow-path helpers (baseline per-layer pipeline) ----
    spikes = None  # current layer's input spike tensor, [128, IC[k], 16, 32] bf16

    def lif_phase(k, V, pk):
        nonlocal spikes
        C = MC[k]
        Vv = V[:].rearrange("p m (b t) -> p m b t", t=T)
        S = spool.tile([128, C, B_CORE, T], BF16, tag="S")
        P = pk.tile([128, C, B_CORE], BF16, tag="P")
        Q = pk.tile([128, C, B_CORE], BF16, tag="Q")
        nc.vector.memset(P[:], 0.0)
        nc.vector.memset(Q[:], 0.0)
        for t in range(T):
            nc.vector.scalar_tensor_tensor(P[:], P[:], 0.9, Q[:], AL.mult, AL.add)
            nc.vector.tensor_scalar(S[:, :, :, t], P[:], 100.0, None, AL.is_gt)
            nc.vector.scalar_tensor_tensor(P[:], P[:], 100.0, P[:], AL.is_le, AL.mult)
            nc.vector.scalar_tensor_tensor(Q[:], Q[:], 0.8, Vv[:, :, :, t], AL.mult, AL.add)
        spikes = S

    def layer_phase(k):
        nonlocal spikes
        M = M_SIZE[k]
        with ExitStack() as ph:
            pk = ph.enter_context(tc.tile_pool(name=f"phase{k + 1}", bufs=1))
            if k == 5:
                V = pk.tile([M, 512], F32, tag="V5")
            else:
                V = pk.tile([128, MC[k], 512], BF16, tag=f"V{k}")

            wk_sb = pk.tile([128, IC[k], OUT_PAD[k]], BF16, tag=f"w{k}")
            nc.sync.dma_start(wk_sb[:], w_aps[k])

            for m in range(MC[k]):
                ps = psum.tile([128, 512], F32, bufs=2)
                for kc in range(IC[k]):
                    nc.tensor.matmul(
                        ps[:M, :],
                        wk_sb[:, kc, m * 128:m * 128 + M],
                        spikes[:, kc, :, :],
                        start=(kc == 0),
                        stop=(kc == IC[k] - 1),
                    )
                j_src = ps[:M, :]
                ij = ijpool.tile([128, 512], BF16)
                nc.vector.tensor_tensor_scan(ij[:M, :], mask08[:M, :], j_src, 0.0, AL.mult, AL.add)
                if k == 5:
                    nc.vector.tensor_tensor_scan(V[:, :], mask09[:M, :], ij[:M, :], 0.0, AL.mult, AL.add)
                else:
                    nc.vector.tensor_tensor_scan(V[:, m, :], mask09[:, :], ij[:, :], 0.0, AL.mult, AL.add)

            if k == 5:
                rmax = pk.tile([M, B_CORE], F32)
                nc.vector.tensor_reduce(
                    rmax[:], V[:].rearrange("p (b t) -> p b t", b=B_CORE),
                    mybir.AxisListType.X, AL.max,
                )
                nc.sync.dma_start(out_ap, rmax[:])
            else:
                lif_phase(k, V, pk)

    with tc.If(sval < 95) as cmp:
        pass
    with cmp.Else():
        emit_masks()
        emit_dense_Y()
        with ExitStack() as phl:
            pl = phl.enter_context(tc.tile_pool(name="lif1", bufs=1))
            # transpose Y to partitions = i for PE expansion against H
            Yt = pl.tile([NMASK, B_CORE, 4, 512], BF16, tag="Yt")
            for b in range(B_CORE):
                nc.sync.dma_start(
                    Yt[:, b, :, :], Ysb[b * NMASK:(b + 1) * NMASK, :, :]
                )
            V0 = pl.tile([128, MC[0], 512], BF16, tag="V0")
            for m in range(MC[0]):
                psv = psum.tile([128, 512], F32, bufs=1)
                for b in range(B_CORE):
                    nc.tensor.matmul(
                        psv[:, b * T:(b + 1) * T],
                        Yt[:, b, m // 4, (m % 4) * 128:(m % 4) * 128 + 128],
                        hc[:],
                        start=True, stop=True,
                    )
                nc.scalar.activation(
                    V0[:, m, :], psv[:], mybir.ActivationFunctionType.Copy, scale=1.0
                )
            lif_phase(0, V0, pl)
        for k in range(1, 6):
            layer_phase(k)


def build_nc():
    from contextlib import ExitStack

    nc = bacc.Bacc("TRN2", debug=False, num_devices=N_CORES)
    xs = nc.dram_tensor("xs", [128, KP0, 2, B_CORE], F32, kind="ExternalInput")
    w_t = [nc.dram_tensor("w0t", [KP0, 128, 2, 2048], FP8, kind="ExternalInput")]
    for k in range(1, 6):
        w_t.append(
            nc.dram_tensor(f"w{k}t", [128, IC[k], OUT_PAD[k]], BF16, kind="ExternalInput")
        )
    hconst = nc.dram_tensor("hconst", [NMASK, T], BF16, kind="ExternalInput")
    aconst = nc.dram_tensor("aconst", [B_CORE * NMASK, B_CORE], BF16, kind="ExternalInput")
    cvconst = nc.dram_tensor("cvconst", [128, KP0 * 2], F32, kind="ExternalInput")
    pioconst = nc.dram_tensor("pioconst", [128, B_CORE], F32, kind="ExternalInput")
    xgath = nc.dram_tensor("xgath", [NFROW, B_CORE], F32, kind="ExternalInput")
    w0gath = nc.dram_tensor("w0gath", [NFROW, 2048], FP8, kind="ExternalInput")
    out = nc.dram_tensor("out", [M_SIZE[5], B_CORE], F32, kind="ExternalOutput")
    dbg = nc.dram_tensor("dbg", [B_CORE, 8], F32, kind="ExternalOutput")

    with tile.TileContext(nc) as tc, ExitStack() as ctx:
        build_body(tc, ctx, nc, xs.ap(), [w.ap() for w in w_t],
                   hconst.ap(), aconst.ap(), out.ap(),
                   cvconst.ap(), pioconst.ap(), xgath.ap(), w0gath.ap(),
                   dbg_ap=dbg.ap())
    nc.compile()
    return nc


def prep_inputs(images, ws):
    """Host-side marshalling: pad/transpose/cast weights, rearrange images."""
    x = np.asarray(images).reshape(128, -1).astype(np.float32)  # [B, 12000]
    xs = np.zeros((128, 12032), np.float32)
    xs[:, :12000] = x
    # [p, kp, j, b] with feature f = (2*kp + j)*128 + p
    xs_r = xs.reshape(128, 47, 2, 128).transpose(3, 1, 2, 0)  # [128p, 47, 2, 128b]
    xs_cores = [
        np.ascontiguousarray(xs_r[:, :, :, c * B_CORE:(c + 1) * B_CORE])
        for c in range(N_CORES)
    ]

    w_prepped = []
    wT0 = np.zeros((12032, 2048), np.float32)
    wT0[:12000, :2000] = np.asarray(ws[0]).T * np.float32(W0_SCALE)
    # [47 kp, 128 p, 2 j, 2048 o]: feature f = (2*kp + j)*128 + p
    w0p = wT0.reshape(47, 2, 128, 2048).transpose(0, 2, 1, 3)
    w_prepped.append(np.ascontiguousarray(w0p.astype(FP8_NP)))
    for k in range(1, 6):
        out_f, in_f = LAYER_SIZES[k]
        wTk = np.zeros((IN_PAD[k], OUT_PAD[k]), np.float32)
        wTk[:in_f, :out_f] = np.asarray(ws[k]).T
        wkp = wTk.reshape(IC[k], 128, OUT_PAD[k]).transpose(1, 0, 2)  # [128p, IC, OUT]
        w_prepped.append(np.ascontiguousarray(wkp.astype(BF16_NP)))

    xgath_cores = []
    for c in range(N_CORES):
        xgc = np.zeros((NFROW, B_CORE), np.float32)
        xgc[:12032] = xs[c * B_CORE:(c + 1) * B_CORE].T
        xgath_cores.append(np.ascontiguousarray(xgc))
    w0gath = np.zeros((NFROW, 2048), np.float32)
    w0gath[:12032] = wT0
    w0gath = np.ascontiguousarray(w0gath.astype(FP8_NP))
    cvmat = np.ascontiguousarray(
        np.tile(np.arange(1, KP0 * 2 + 1, dtype=np.float32)[None, :], (128, 1))
    )
    piomat = np.ascontiguousarray(
        np.tile(np.arange(128, dtype=np.float32)[:, None], (1, B_CORE))
    )
    _Bs, _D, H = _mask_tables()
    hmat = np.ascontiguousarray(H.astype(BF16_NP))  # [4, 32]
    A = np.abs(H).max(1)  # per-channel max_t |H|
    amat = np.zeros((B_CORE * NMASK, B_CORE), np.float32)
    for b in range(B_CORE):
        for i in range(NMASK):
            amat[b * NMASK + i, b] = A[i]
    amat = np.ascontiguousarray(amat.astype(BF16_NP))
    return xs_cores, w_prepped, hmat, amat, cvmat, piomat, xgath_cores, w0gath


_NC_CACHE = {}


def kernel(images, w0, w1, w2, w3, w4, w5):
    global LAST_EXEC_TIME_NS
    ws = [w0, w1, w2, w3, w4, w5]
    (xs_cores, w_prepped, hmat, amat, cvmat, piomat,
     xgath_cores, w0gath) = prep_inputs(images, ws)

    trace = os.environ.get("KERNEL_TRACE", "0") == "1"
    if trace:
        _install_ntff_hook()

    if "nc" not in _NC_CACHE:
        _NC_CACHE["nc"] = build_nc()
    nc = _NC_CACHE["nc"]

    in_maps = []
    for c in range(N_CORES):
        m = {"xs": xs_cores[c], "w0t": w_prepped[0], "hconst": hmat, "aconst": amat,
             "cvconst": cvmat, "pioconst": piomat, "xgath": xgath_cores[c],
             "w0gath": w0gath}
        for k in range(1, 6):
            m[f"w{k}t"] = w_prepped[k]
        in_maps.append(m)

    res = run_bass_kernel_spmd(
        nc, in_maps, core_ids=list(range(N_CORES)), trace=trace
    )
    LAST_EXEC_TIME_NS = res.exec_time_ns
    _NC_CACHE["res"] = res

    # out[c] is [16 feats, 16 batch]; valid feats :10; logits = max_t(V5)/10
    logits = np.concatenate(
        [np.asarray(res.results[c]["out"])[:10, :].T for c in range(N_CORES)], axis=0
    ).astype(np.float32) / np.float32(10.0)
    mx = logits.max(axis=1, keepdims=True)
    sh = logits - mx
    out = sh - np.log(np.exp(sh).sum(axis=1, keepdims=True))
    return out.astype(np.float32)
